# revision 32
# baseline (speedup 1.0000x reference)
"""MLA (Multi-head Latent Attention) Bass/Tile kernel for 8 Trainium2 NeuronCores.

Problem: nn_MultiHeadLatentAttention_81707457839331
  B=2, S=2048, HID=2048, NH=16 heads, NOPE=128, ROPE=64, VD=128, QKD=192,
  KVR=512, QR=1536, fp32.

Sharding (single NEFF, SPMD on 8 cores):
  core c -> batch b = c//4, head group g = c%4 (4 heads each).
  The down-projections are SEQUENCE-PARALLEL within each 4-core batch group:
  core (b, g) computes the fused down-proj (q-latent + c_kv + roped-key rows)
  for its 512-token slice only, applies the RMSNorm scales locally (it owns
  all features of its tokens), writes the normalized latents to DRAM, and a
  4-rank AllGather rebuilds the full-S latents on every core of the group.
  q_up / kv_up / attention / o_proj are head-sharded as before. Each core
  emits a partial o_proj output [S, HID]; the host sums 4 partials per batch.

On-device layout: everything stays in [feature, token] layouts so no
on-device transposes are needed. All matmul operands are bf16 (full PE rate,
half the DMA/SBUF footprint, 2x DVE rate); PSUM accumulation stays fp32.
RMSNorm scales are folded into the latents before the AllGather (and SCALE
into the q-norm), RoPE rotate_half is folded into host-rotated weight
columns, the shared roped key is materialized as [k;0]/[0;k] so rope score
matmuls run at full K=128, softmax is unnormalized with bf16 exp tiles, and
the probability sums ride DVE adds + a GpSimd partition_all_reduce.
Phases C (q_up), D (attention) and F (o_proj) interleave per 512-token tile
to keep the PE continuously busy; K/V/Q/O all stay SBUF-resident.
"""

import numpy as np
import ml_dtypes

import concourse.bass as bass
import concourse.bass_isa as bass_isa
import concourse.mybir as mybir
import concourse.tile as tile
from concourse import bacc
from concourse.bass import ds, ts
from concourse.bass_utils import run_bass_kernel_spmd

F32 = mybir.dt.float32
BF16 = mybir.dt.bfloat16
AF = mybir.ActivationFunctionType
BF16_NP = ml_dtypes.bfloat16

B, S, HID, NH = 2, 2048, 2048, 16
NOPE, ROPE, VD = 128, 64, 128
QKD = NOPE + ROPE
KVR, QR = 512, 1536
EPS = 1e-6
SCALE = QKD ** (-0.5)
P = 128

NHC = HID // P            # 16 hidden chunks
NQC = QR // P             # 12 q-latent chunks
NFC = 17                  # down-proj output chunks (12 qlat + 4 ckv + rope-dup)
NKC = KVR // P            # 4 ckv chunks
NTT = S // 512            # 4 token tiles of 512
NTC = S // P              # 16 token chunks of 128
NDQ = 6                   # q_up output chunks: 4 nope + 2 rope-pairs
NEG = -1e30
GROUPS = [[0, 1, 2, 3], [4, 5, 6, 7]]


def _emit(tc):
    nc = tc.nc
    hid_in = nc.dram_tensor("hid", [P, NHC, 512], BF16, kind="ExternalInput").ap()
    cos_in = nc.dram_tensor("cos2", [P, S], BF16, kind="ExternalInput").ap()
    sin_in = nc.dram_tensor("sin2", [P, S], BF16, kind="ExternalInput").ap()
    wd_in = nc.dram_tensor("wd", [NFC, P, NHC, P], BF16, kind="ExternalInput").ap()
    wqup_in = nc.dram_tensor("wqup", [P, NQC, NDQ * P], BF16,
                             kind="ExternalInput").ap()
    wkup_in = nc.dram_tensor("wkup", [P, NKC, 512], BF16, kind="ExternalInput").ap()
    wvup_in = nc.dram_tensor("wvup", [P, NKC, 512], BF16, kind="ExternalInput").ap()
    wo_in = nc.dram_tensor("wo", [P, 4, HID], BF16, kind="ExternalInput").ap()
    rotm_in = nc.dram_tensor("rotm", [P, P], BF16, kind="ExternalInput").ap()
    out_d = nc.dram_tensor("out", [S, HID], F32, kind="ExternalOutput").ap()

    with (
        tc.tile_pool(name="const", bufs=1) as constp,
        tc.tile_pool(name="persist", bufs=1) as pers,
        tc.tile_pool(name="dram", bufs=1, space="DRAM") as dramp,
    ):
        eps_kv = constp.tile([P, 1], F32)
        nc.vector.memset(eps_kv, EPS)
        eps_q = constp.tile([P, 1], F32)
        nc.vector.memset(eps_q, EPS / (SCALE * SCALE))
        # 4 causal additive masks: mask_k[p, x] = 0 if x - p - 128k >= 0 else -1e30
        masks = []
        for k in range(4):
            m = constp.tile([P, 512], F32, name=f"mask{k}")
            nc.gpsimd.memset(m, 0.0)
            nc.gpsimd.affine_select(
                out=m, in_=m, pattern=[[1, 512]],
                compare_op=mybir.AluOpType.is_ge, fill=NEG,
                base=-128 * k, channel_multiplier=-1,
            )
            masks.append(m)
        cos_sb = constp.tile([P, S], BF16)
        sin_sb = constp.tile([P, S], BF16)
        nc.scalar.dma_start(cos_sb, cos_in)
        nc.scalar.dma_start(sin_sb, sin_in)
        # rotate_half as a K=128 matmul: rot(x) = rotm^T @ x (per 64-block)
        rotm = constp.tile([P, P], BF16)
        nc.scalar.dma_start(rotm, rotm_in)

        # SBUF-resident key/value/query/output tensors (bf16)
        kt_sb = pers.tile([P, 4, S], BF16)     # 4 heads' k_nope.T
        kre_sb = pers.tile([P, S], BF16)       # [k_roped; 0]
        kro_sb = pers.tile([P, S], BF16)       # [0; k_roped]
        v_sb = pers.tile([P, NTC, 512], BF16)  # V in [token, 4*VD]
        qT = pers.tile([P, 6, S], BF16)        # 4 nope + 2 roped pairs
        o_sb = pers.tile([P, 4, S], BF16)      # attention out, head-major

        # DRAM staging for the sequence-parallel down-proj + AllGather.
        # q latents are gathered RAW in two halves (pipelined collectives);
        # their RMSNorm happens in phase C, fused into the psum evictions.
        latq_in_a = dramp.tile([P, 6, 512], BF16)
        latq_in_b = dramp.tile([P, 6, 512], BF16)
        latkv_in = dramp.tile([P, 6, 512], BF16)
        latq_all_a = dramp.tile([NTT, P, 6, 512], BF16)
        latq_all_b = dramp.tile([NTT, P, 6, 512], BF16)
        latkv_all = dramp.tile([NTT, P, 6, 512], BF16)

        # ---------- Phase A: seq-parallel fused down-proj + norms + AG ----------
        with (
            tc.tile_pool(name="pa_hid", bufs=1) as ph,
            tc.tile_pool(name="pa_lat", bufs=1) as plat,
            tc.tile_pool(name="pa_w", bufs=3) as pw,
            tc.tile_pool(name="pa_row", bufs=3) as prow,
            tc.tile_pool(name="pa_tmp", bufs=2) as pat,
            tc.tile_pool(name="pa_ps", bufs=3, space="PSUM") as pps,
        ):
            with nc.named_scope("phaseA"):
                hid_sb = ph.tile([P, NHC, 512], BF16)
                for q in range(4):
                    nc.sync.dma_start(hid_sb[:, 4 * q:4 * q + 4, :],
                                      hid_in[:, 4 * q:4 * q + 4, :])
                ckv_sb = plat.tile([P, NKC, 512], BF16)

                def dp_row(fc, ps_name):
                    w_sb = pw.tile([P, NHC, P], BF16, name="wslice")
                    nc.sync.dma_start(w_sb, wd_in[fc])
                    ps = pps.tile([P, 512], F32, name=ps_name)
                    for hc in range(NHC):
                        nc.tensor.matmul(
                            ps, w_sb[:, hc, :], hid_sb[:, hc, :],
                            start=(hc == 0), stop=(hc == NHC - 1),
                        )
                    return ps

                # ckv rows, then their norm while rope/rot rows run on PE
                for fc in range(12, 16):
                    ps = dp_row(fc, "aps")
                    nc.vector.tensor_copy(ckv_sb[:, fc - 12, :], ps)
                acc = pat.tile([P, 512], F32, name="aacc")
                nc.scalar.square(acc, ckv_sb[:, 0, :])
                for fc in range(1, NKC):
                    sq = pat.tile([P, 512], F32, name="asq")
                    nc.scalar.square(sq, ckv_sb[:, fc, :])
                    nc.vector.tensor_add(acc, acc, sq)
                ar = pat.tile([P, 512], F32, name="aar")
                nc.gpsimd.partition_all_reduce(ar, acc, channels=P,
                                               reduce_op=bass_isa.ReduceOp.add)
                nc.scalar.activation(ar, ar, AF.Sqrt, bias=eps_kv, scale=1.0 / KVR)
                nc.vector.reciprocal(ar, ar)
                ps = dp_row(16, "aps")
                row = prow.tile([P, 512], BF16, name="arow")
                nc.vector.tensor_copy(row, ps)
                nc.sync.dma_start(latkv_in[:, 4, :], row)
                ps_rot = pps.tile([P, 512], F32, name="apsrot")
                nc.tensor.matmul(ps_rot, rotm, row, start=True, stop=True)
                rrow = prow.tile([P, 512], BF16, name="arrow")
                nc.vector.tensor_copy(rrow, ps_rot)
                nc.sync.dma_start(latkv_in[:, 5, :], rrow)
                for fc in range(NKC):
                    crow = prow.tile([P, 512], BF16, name="acrow")
                    nc.vector.tensor_mul(crow, ckv_sb[:, fc, :], ar)
                    nc.sync.dma_start(latkv_in[:, fc, :], crow)
                nc.gpsimd.collective_compute(
                    "AllGather", mybir.AluOpType.bypass, replica_groups=GROUPS,
                    ins=[latkv_in[:]], outs=[latkv_all[:]],
                )
                # raw q rows; two pipelined AllGather halves
                for half, lat_in, lat_all in ((0, latq_in_a, latq_all_a),
                                              (1, latq_in_b, latq_all_b)):
                    for fc in range(6 * half, 6 * half + 6):
                        ps = dp_row(fc, "aps")
                        qrow = prow.tile([P, 512], BF16, name="aqrow")
                        nc.vector.tensor_copy(qrow, ps)
                        nc.sync.dma_start(lat_in[:, fc - 6 * half, :], qrow)
                    nc.gpsimd.collective_compute(
                        "AllGather", mybir.AluOpType.bypass,
                        replica_groups=GROUPS,
                        ins=[lat_in[:]], outs=[lat_all[:]],
                    )

        # ---------- Phase B: rope-k + kv_up (latents already normalized) ----------
        with (
            tc.tile_pool(name="pb", bufs=1) as pb,
            tc.tile_pool(name="pb_tmp", bufs=2) as pbt,
            tc.tile_pool(name="pb_ps", bufs=3, space="PSUM") as pps2,
        ):
            with nc.named_scope("phaseB"):
                wk_sb = pb.tile([P, NKC, 512], BF16)
                wv_sb = pb.tile([P, NKC, 512], BF16)
                nc.scalar.dma_start(wk_sb, wkup_in)
                nc.scalar.dma_start(wv_sb, wvup_in)
                kv_sb = pb.tile([P, 6, S], BF16)
                for tt in range(NTT):
                    nc.sync.dma_start(kv_sb[:, :, ts(tt, 512)], latkv_all[tt])
                # roped shared key -> [k;0] and [0;k]
                krd = pb.tile([P, S], BF16)
                t1 = pbt.tile([P, S], BF16, name="bt1")
                nc.vector.tensor_mul(t1, kv_sb[:, 4, :], cos_sb)
                nc.vector.tensor_mul(krd, kv_sb[:, 5, :], sin_sb)
                nc.vector.tensor_add(krd, krd, t1)
                nc.vector.tensor_copy(kre_sb, krd)
                nc.vector.tensor_scalar_mul(kre_sb[64:128, :], kre_sb[64:128, :], 0.0)
                nc.vector.tensor_copy(kro_sb, krd)
                nc.vector.tensor_scalar_mul(kro_sb[0:64, :], kro_sb[0:64, :], 0.0)
                # kv_up: kt (4 head chunks) and V (16 token chunks)
                for d in range(4):
                    for tt in range(NTT):
                        ps = pps2.tile([P, 512], F32, name="bps")
                        for fc in range(NKC):
                            nc.tensor.matmul(
                                ps, wk_sb[:, fc, ds(d * P, P)],
                                kv_sb[:, fc, ts(tt, 512)],
                                start=(fc == 0), stop=(fc == NKC - 1),
                            )
                        nc.scalar.activation(kt_sb[:, d, ts(tt, 512)], ps, AF.Copy)
                for tch in range(NTC):
                    ps = pps2.tile([P, 512], F32, name="bpsv")
                    for fc in range(NKC):
                        nc.tensor.matmul(
                            ps, kv_sb[:, fc, ds(tch * P, P)], wv_sb[:, fc, :],
                            start=(fc == 0), stop=(fc == NKC - 1),
                        )
                    nc.scalar.activation(v_sb[:, tch, :], ps, AF.Copy)

        # ---------- Phases C+D+F interleaved per 512-token tile ----------
        with (
            tc.tile_pool(name="pc_w", bufs=1) as pcw,
            tc.tile_pool(name="pc_slab", bufs=2) as pcs,
            tc.tile_pool(name="pc_tmp", bufs=2) as pct,
            tc.tile_pool(name="pc_ps", bufs=2, space="PSUM") as pps3,
            tc.tile_pool(name="pd_e", bufs=3) as pde,
            tc.tile_pool(name="pd_t", bufs=2) as pdt,
            tc.tile_pool(name="pd_psc", bufs=3, space="PSUM") as pdsc,
            tc.tile_pool(name="pd_pso", bufs=2, space="PSUM") as pdo,
            tc.tile_pool(name="pf_w", bufs=1) as pfw,
            tc.tile_pool(name="pf_row", bufs=2) as pfr,
            tc.tile_pool(name="pf_ps", bufs=1, space="PSUM") as pfp,
        ):
            wq_sb = pcw.tile([P, NQC, NDQ * P], BF16)
            nc.sync.dma_start(wq_sb, wqup_in)
            wo_sb = pfw.tile([P, 4, HID], BF16)
            nc.scalar.dma_start(wo_sb, wo_in)

            def load_slab(i):
                slab = pcs.tile([P, NQC, 512], BF16, name="qslabin")
                nc.sync.dma_start(slab[:, 0:6, :], latq_all_a[i])
                nc.sync.dma_start(slab[:, 6:12, :], latq_all_b[i])
                return slab

            slabs = {0: load_slab(0), 1: load_slab(1)}
            for i in range(NTT):
                tts = ts(i, 512)
                # --- C(i): q_up + q-norm (fused into evictions) + rope-q
                with nc.named_scope(f"phaseC{i}"):
                    if i + 2 < NTT:
                        slabs[i + 2] = load_slab(i + 2)
                    slab = slabs.pop(i)
                    # rq = SCALE / rms(qlat), from the raw gathered latents
                    acc = pct.tile([P, 512], BF16, name="cacc")
                    nc.scalar.square(acc, slab[:, 0, :])
                    for fc in range(1, NQC):
                        sq = pct.tile([P, 512], BF16, name="csq")
                        nc.scalar.square(sq, slab[:, fc, :])
                        nc.vector.tensor_add(acc, acc, sq)
                    rq_b = pcs.tile([P, 512], F32, name="crqb")
                    nc.gpsimd.partition_all_reduce(
                        rq_b, acc, channels=P, reduce_op=bass_isa.ReduceOp.add)
                    nc.scalar.activation(rq_b, rq_b, AF.Sqrt, bias=eps_q,
                                         scale=1.0 / (QR * SCALE * SCALE))
                    nc.vector.reciprocal(rq_b, rq_b)
                    rp = []
                    for d in range(NDQ):
                        ps = pps3.tile([P, 512], F32, name="cps")
                        for fc in range(NQC):
                            nc.tensor.matmul(
                                ps, wq_sb[:, fc, ds(d * P, P)], slab[:, fc, :],
                                start=(fc == 0), stop=(fc == NQC - 1),
                            )
                        if d < 4:
                            nc.vector.tensor_mul(qT[:, d, tts], ps, rq_b)
                        else:
                            r = pct.tile([P, 512], BF16, name=f"rp{d - 4}")
                            nc.vector.tensor_copy(r, ps)
                            rp.append(r)
                    for pr in range(2):
                        ps = pps3.tile([P, 512], F32, name="cps")
                        nc.tensor.matmul(ps, rotm, rp[pr], start=True, stop=True)
                        rr = pct.tile([P, 512], BF16, name="crr")
                        t1 = pct.tile([P, 512], BF16, name="ct1")
                        nc.vector.tensor_copy(rr, ps)
                        nc.vector.tensor_mul(t1, rp[pr], cos_sb[:, tts])
                        nc.vector.tensor_mul(rr, rr, sin_sb[:, tts])
                        nc.vector.tensor_add(t1, t1, rr)
                        nc.vector.tensor_mul(qT[:, 4 + pr, tts], t1, rq_b)

                # --- D(i): attention rows for queries in this tile
                with nc.named_scope(f"phaseD{i}"):
                    jmax = 4 * i + 3
                    for h in range(4):
                        krop = kre_sb if h % 2 == 0 else kro_sb
                        qp = qT[:, 4 + h // 2, :]
                        ps_o = pdo.tile([P, 512], F32, name="pso")
                        eacc = pdt.tile([P, 512], BF16, name="eacc")
                        for jc in range(jmax + 1):
                            # diagonal tiles only touch the causally-valid
                            # query range [128m, 512); the rest is never read
                            m = jc - 4 * i
                            lo = 128 * m if m > 0 else 0
                            L = 512 - lo
                            cq = ds(lo, L)
                            ps_sc = pdsc.tile([P, 512], F32, name="psc")
                            nc.tensor.matmul(
                                ps_sc[:, cq], kt_sb[:, h, ds(jc * P, P)],
                                qT[:, h, ds(512 * i + lo, L)],
                                start=True, stop=False)
                            nc.tensor.matmul(
                                ps_sc[:, cq], krop[:, ds(jc * P, P)],
                                qp[:, ds(512 * i + lo, L)],
                                start=False, stop=True)
                            if m >= 0:
                                nc.vector.tensor_add(
                                    ps_sc[:, ds(lo, P)], ps_sc[:, ds(lo, P)],
                                    masks[m][:, ds(lo, P)])
                            et = pde.tile([P, 512], BF16, name="et")
                            nc.scalar.activation(et[:, cq], ps_sc[:, cq], AF.Exp)
                            nc.tensor.matmul(
                                ps_o[:, cq], v_sb[:, jc, ds(h * P, P)],
                                et[:, cq], start=(jc == 0), stop=(jc == jmax))
                            if jc == 0:
                                nc.gpsimd.tensor_copy(eacc, et)
                            else:
                                nc.vector.tensor_add(
                                    eacc[:, cq], eacc[:, cq], et[:, cq])
                        ar = pdt.tile([P, 512], F32, name="dar")
                        nc.gpsimd.partition_all_reduce(
                            ar, eacc, channels=P,
                            reduce_op=bass_isa.ReduceOp.add)
                        nc.vector.reciprocal(ar, ar)
                        nc.vector.tensor_mul(o_sb[:, h, tts], ps_o, ar)

                # --- F(i): o_proj partial for this token tile
                with nc.named_scope(f"phaseF{i}"):
                    for tch in range(4 * i, 4 * i + 4):
                        orow = pfr.tile([P, HID], F32, name="orow")
                        for ct in range(4):
                            ps = pfp.tile([P, 512], F32, name="fps")
                            for hh in range(4):
                                nc.tensor.matmul(
                                    ps, o_sb[:, hh, ds(tch * P, P)],
                                    wo_sb[:, hh, ts(ct, 512)],
                                    start=(hh == 0), stop=(hh == 3),
                                )
                            nc.any.tensor_copy(orow[:, ts(ct, 512)], ps)
                        nc.scalar.dma_start(out_d[ds(tch * P, P), :], orow)


_NC_CACHE = None


def _build_nc():
    global _NC_CACHE
    if _NC_CACHE is None:
        nc = bacc.Bacc("TRN2", target_bir_lowering=False, debug=False,
                       num_devices=8)
        with tile.TileContext(nc) as tc:
            _emit(tc)
        nc.compile()
        _NC_CACHE = nc
    return _NC_CACHE


def _shard_inputs(hidden_states, cos, sin, Wq_down, q_gamma, Wq_up,
                  Wkv_down, kv_gamma, Wkv_up, Wo):
    f32 = np.float32
    hid = np.ascontiguousarray(np.asarray(hidden_states, dtype=f32))
    cos = np.asarray(cos, dtype=f32)
    sin = np.asarray(sin, dtype=f32)
    Wqd = np.asarray(Wq_down, dtype=f32)
    Wkd = np.asarray(Wkv_down, dtype=f32)
    qg = np.asarray(q_gamma, dtype=f32)
    kvg = np.asarray(kv_gamma, dtype=f32)
    Wqu = np.asarray(Wq_up, dtype=f32) * qg[None, :]
    Wku = np.asarray(Wkv_up, dtype=f32) * kvg[None, :]
    Wo = np.asarray(Wo, dtype=f32)

    def bf(x):
        return np.ascontiguousarray(x).astype(BF16_NP)

    # shared: combined down-proj weight (rot rows come from the rotm matmul)
    WqdT = Wqd.T                                   # [HID, QR]
    WckvT = Wkd[:KVR].T                            # [HID, KVR]
    krope = Wkd[KVR:].T                            # [HID, 64]
    WdT = np.concatenate([WqdT, WckvT, krope, krope], 1)  # [HID, 2176]
    wd = bf(WdT.reshape(NHC, P, NFC, P).transpose(2, 1, 0, 3))  # [17,128,16,128]
    # rotate_half operator, stationary layout (rotm[p, j] = R2^T)
    r64 = np.zeros((64, 64), dtype=np.float32)
    r64[0:32, 32:64] = np.eye(32)
    r64[32:64, 0:32] = -np.eye(32)
    rotm = np.zeros((P, P), dtype=np.float32)
    rotm[0:64, 0:64] = r64
    rotm[64:128, 64:128] = r64
    rotm = bf(rotm)

    per_batch = []
    for b in range(B):
        h_sw = hid[b].T.reshape(NHC, P, S).transpose(1, 0, 2)  # [128, 16, 2048]
        cT = cos[b].T                               # [64, S]
        sT = sin[b].T
        cos2 = bf(np.concatenate([cT, cT], 0))
        sin2 = bf(np.concatenate([sT, sT], 0))
        per_batch.append((h_sw, cos2, sin2))

    per_group = []
    for g in range(4):
        bn, br = [], []
        for hl in range(4):
            h = 4 * g + hl
            blk = Wqu[h * QKD:(h + 1) * QKD]       # [192, QR]
            bn.append(blk[:NOPE])
            br.append(blk[NOPE:])
        cols = bn + [np.concatenate([br[0], br[1]], 0),
                     np.concatenate([br[2], br[3]], 0)]
        WquT = np.concatenate(cols, 0).T           # [QR, 768]
        wqup = bf(WquT.reshape(NQC, P, NDQ * P).transpose(1, 0, 2))
        kb, vb = [], []
        for hl in range(4):
            h = 4 * g + hl
            blk = Wku[h * (NOPE + VD):(h + 1) * (NOPE + VD)]
            kb.append(blk[:NOPE])
            vb.append(blk[NOPE:])
        WkuT = np.concatenate(kb, 0).T             # [KVR, 512]
        WvuT = np.concatenate(vb, 0).T
        wkup = bf(WkuT.reshape(NKC, P, 512).transpose(1, 0, 2))
        wvup = bf(WvuT.reshape(NKC, P, 512).transpose(1, 0, 2))
        WoT = Wo[:, g * 512:(g + 1) * 512].T       # [512, HID]
        wo = bf(WoT.reshape(4, P, HID).transpose(1, 0, 2))
        per_group.append((wqup, wkup, wvup, wo))

    in_maps = []
    for c in range(8):
        b, g = c // 4, c % 4
        h_sw, cos2, sin2 = per_batch[b]
        wqup, wkup, wvup, wo = per_group[g]
        in_maps.append({
            "hid": bf(h_sw[:, :, 512 * g:512 * (g + 1)]),
            "cos2": cos2, "sin2": sin2, "wd": wd, "rotm": rotm,
            "wqup": wqup, "wkup": wkup, "wvup": wvup, "wo": wo,
        })
    return in_maps


def kernel(hidden_states, cos, sin, Wq_down, q_gamma, Wq_up,
           Wkv_down, kv_gamma, Wkv_up, Wo, _trace=False):
    nc = _build_nc()
    in_maps = _shard_inputs(hidden_states, cos, sin, Wq_down, q_gamma, Wq_up,
                            Wkv_down, kv_gamma, Wkv_up, Wo)
    res = run_bass_kernel_spmd(nc, in_maps, core_ids=list(range(8)),
                               trace=_trace)
    out = np.zeros((B, S, HID), dtype=np.float32)
    for c in range(8):
        out[c // 4] += res.results[c]["out"]
    if _trace:
        kernel.last_results = res
    return out


# revision 37
# speedup vs baseline: 1.0139x; 1.0139x over previous
"""MLA (Multi-head Latent Attention) Bass/Tile kernel for 8 Trainium2 NeuronCores.

Problem: nn_MultiHeadLatentAttention_81707457839331
  B=2, S=2048, HID=2048, NH=16 heads, NOPE=128, ROPE=64, VD=128, QKD=192,
  KVR=512, QR=1536, fp32.

Sharding (single NEFF, SPMD on 8 cores):
  core c -> batch b = c//4, head group g = c%4 (4 heads each).
  The down-projections are SEQUENCE-PARALLEL within each 4-core batch group:
  core (b, g) computes the fused down-proj (q-latent + c_kv + roped-key rows)
  for its 512-token slice only, applies the RMSNorm scales locally (it owns
  all features of its tokens), writes the normalized latents to DRAM, and a
  4-rank AllGather rebuilds the full-S latents on every core of the group.
  q_up / kv_up / attention / o_proj are head-sharded as before. Each core
  emits a partial o_proj output [S, HID]; the host sums 4 partials per batch.

On-device layout: everything stays in [feature, token] layouts so no
on-device transposes are needed. All matmul operands are bf16 (full PE rate,
half the DMA/SBUF footprint, 2x DVE rate); PSUM accumulation stays fp32.
RMSNorm scales are folded into the latents before the AllGather (and SCALE
into the q-norm), RoPE rotate_half is folded into host-rotated weight
columns, the shared roped key is materialized as [k;0]/[0;k] so rope score
matmuls run at full K=128, softmax is unnormalized with bf16 exp tiles, and
the probability sums ride DVE adds + a GpSimd partition_all_reduce.
Phases C (q_up), D (attention) and F (o_proj) interleave per 512-token tile
to keep the PE continuously busy; K/V/Q/O all stay SBUF-resident.
"""

import numpy as np
import ml_dtypes

import concourse.bass as bass
import concourse.bass_isa as bass_isa
import concourse.mybir as mybir
import concourse.tile as tile
from concourse import bacc
from concourse.bass import ds, ts
from concourse.bass_utils import run_bass_kernel_spmd

F32 = mybir.dt.float32
BF16 = mybir.dt.bfloat16
AF = mybir.ActivationFunctionType
BF16_NP = ml_dtypes.bfloat16

B, S, HID, NH = 2, 2048, 2048, 16
NOPE, ROPE, VD = 128, 64, 128
QKD = NOPE + ROPE
KVR, QR = 512, 1536
EPS = 1e-6
SCALE = QKD ** (-0.5)
P = 128

NHC = HID // P            # 16 hidden chunks
NQC = QR // P             # 12 q-latent chunks
NFC = 17                  # down-proj output chunks (12 qlat + 4 ckv + rope-dup)
NKC = KVR // P            # 4 ckv chunks
NTT = S // 512            # 4 token tiles of 512
NTC = S // P              # 16 token chunks of 128
NDQ = 6                   # q_up output chunks: 4 nope + 2 rope-pairs
NEG = -1e30
GROUPS = [[0, 1, 2, 3], [4, 5, 6, 7]]


def _emit(tc):
    nc = tc.nc
    hid_in = nc.dram_tensor("hid", [P, NHC, 512], BF16, kind="ExternalInput").ap()
    cos_in = nc.dram_tensor("cos2", [P, S], BF16, kind="ExternalInput").ap()
    sin_in = nc.dram_tensor("sin2", [P, S], BF16, kind="ExternalInput").ap()
    wd_in = nc.dram_tensor("wd", [NFC, P, NHC, P], BF16, kind="ExternalInput").ap()
    wqup_in = nc.dram_tensor("wqup", [P, NQC, NDQ * P], BF16,
                             kind="ExternalInput").ap()
    wkup_in = nc.dram_tensor("wkup", [P, NKC, 512], BF16, kind="ExternalInput").ap()
    wvup_in = nc.dram_tensor("wvup", [P, NKC, 512], BF16, kind="ExternalInput").ap()
    wo_in = nc.dram_tensor("wo", [P, 4, HID], BF16, kind="ExternalInput").ap()
    rotm_in = nc.dram_tensor("rotm", [P, P], BF16, kind="ExternalInput").ap()
    out_d = nc.dram_tensor("out", [S, HID], BF16, kind="ExternalOutput").ap()

    with (
        tc.tile_pool(name="const", bufs=1) as constp,
        tc.tile_pool(name="persist", bufs=1) as pers,
        tc.tile_pool(name="dram", bufs=1, space="DRAM") as dramp,
    ):
        eps_kv = constp.tile([P, 1], F32)
        nc.vector.memset(eps_kv, EPS)
        eps_q = constp.tile([P, 1], F32)
        nc.vector.memset(eps_q, EPS / (SCALE * SCALE))
        # 4 causal additive masks: mask_k[p, x] = 0 if x - p - 128k >= 0 else -1e30
        masks = []
        for k in range(4):
            m = constp.tile([P, 512], F32, name=f"mask{k}")
            nc.gpsimd.memset(m, 0.0)
            nc.gpsimd.affine_select(
                out=m, in_=m, pattern=[[1, 512]],
                compare_op=mybir.AluOpType.is_ge, fill=NEG,
                base=-128 * k, channel_multiplier=-1,
            )
            masks.append(m)
        cos_sb = constp.tile([P, S], BF16)
        sin_sb = constp.tile([P, S], BF16)
        nc.scalar.dma_start(cos_sb, cos_in)
        nc.scalar.dma_start(sin_sb, sin_in)
        # rotate_half as a K=128 matmul: rot(x) = rotm^T @ x (per 64-block)
        rotm = constp.tile([P, P], BF16)
        nc.scalar.dma_start(rotm, rotm_in)

        # SBUF-resident key/value/query/output tensors (bf16)
        kt_sb = pers.tile([P, 4, S], BF16)     # 4 heads' k_nope.T
        kre_sb = pers.tile([P, S], BF16)       # [k_roped; 0]
        kro_sb = pers.tile([P, S], BF16)       # [0; k_roped]
        v_sb = pers.tile([P, NTC, 512], BF16)  # V in [token, 4*VD]
        qT = pers.tile([P, 6, S], BF16)        # 4 nope + 2 roped pairs
        o_sb = pers.tile([P, 4, S], BF16)      # attention out, head-major

        # DRAM staging for the sequence-parallel down-proj + AllGather.
        # q latents are gathered RAW in two halves (pipelined collectives);
        # their RMSNorm happens in phase C, fused into the psum evictions.
        latq_in_a = dramp.tile([P, 6, 512], BF16)
        latq_in_b = dramp.tile([P, 6, 512], BF16)
        latkv_in = dramp.tile([P, 6, 512], BF16)
        latq_all_a = dramp.tile([NTT, P, 6, 512], BF16)
        latq_all_b = dramp.tile([NTT, P, 6, 512], BF16)
        latkv_all = dramp.tile([NTT, P, 6, 512], BF16)

        # ---------- Phase A: seq-parallel fused down-proj + norms + AG ----------
        with (
            tc.tile_pool(name="pa_hid", bufs=1) as ph,
            tc.tile_pool(name="pa_lat", bufs=1) as plat,
            tc.tile_pool(name="pa_w", bufs=3) as pw,
            tc.tile_pool(name="pa_row", bufs=3) as prow,
            tc.tile_pool(name="pa_tmp", bufs=2) as pat,
            tc.tile_pool(name="pa_ps", bufs=3, space="PSUM") as pps,
        ):
            with nc.named_scope("phaseA"):
                hid_sb = ph.tile([P, NHC, 512], BF16)
                for q in range(4):
                    nc.sync.dma_start(hid_sb[:, 4 * q:4 * q + 4, :],
                                      hid_in[:, 4 * q:4 * q + 4, :])
                ckv_sb = plat.tile([P, NKC, 512], BF16)

                def dp_row(fc, ps_name):
                    w_sb = pw.tile([P, NHC, P], BF16, name="wslice")
                    nc.sync.dma_start(w_sb, wd_in[fc])
                    ps = pps.tile([P, 512], F32, name=ps_name)
                    for hc in range(NHC):
                        nc.tensor.matmul(
                            ps, w_sb[:, hc, :], hid_sb[:, hc, :],
                            start=(hc == 0), stop=(hc == NHC - 1),
                        )
                    return ps

                # ckv rows, then their norm while rope/rot rows run on PE
                for fc in range(12, 16):
                    ps = dp_row(fc, "aps")
                    nc.vector.tensor_copy(ckv_sb[:, fc - 12, :], ps)
                acc = pat.tile([P, 512], F32, name="aacc")
                nc.scalar.square(acc, ckv_sb[:, 0, :])
                for fc in range(1, NKC):
                    sq = pat.tile([P, 512], F32, name="asq")
                    nc.scalar.square(sq, ckv_sb[:, fc, :])
                    nc.vector.tensor_add(acc, acc, sq)
                ar = pat.tile([P, 512], F32, name="aar")
                nc.gpsimd.partition_all_reduce(ar, acc, channels=P,
                                               reduce_op=bass_isa.ReduceOp.add)
                nc.scalar.activation(ar, ar, AF.Sqrt, bias=eps_kv, scale=1.0 / KVR)
                nc.vector.reciprocal(ar, ar)
                ps = dp_row(16, "aps")
                row = prow.tile([P, 512], BF16, name="arow")
                nc.vector.tensor_copy(row, ps)
                nc.sync.dma_start(latkv_in[:, 4, :], row)
                ps_rot = pps.tile([P, 512], F32, name="apsrot")
                nc.tensor.matmul(ps_rot, rotm, row, start=True, stop=True)
                rrow = prow.tile([P, 512], BF16, name="arrow")
                nc.vector.tensor_copy(rrow, ps_rot)
                nc.sync.dma_start(latkv_in[:, 5, :], rrow)
                for fc in range(NKC):
                    crow = prow.tile([P, 512], BF16, name="acrow")
                    nc.vector.tensor_mul(crow, ckv_sb[:, fc, :], ar)
                    nc.sync.dma_start(latkv_in[:, fc, :], crow)
                nc.gpsimd.collective_compute(
                    "AllGather", mybir.AluOpType.bypass, replica_groups=GROUPS,
                    ins=[latkv_in[:]], outs=[latkv_all[:]],
                )
                # raw q rows; two pipelined AllGather halves
                for half, lat_in, lat_all in ((0, latq_in_a, latq_all_a),
                                              (1, latq_in_b, latq_all_b)):
                    for fc in range(6 * half, 6 * half + 6):
                        ps = dp_row(fc, "aps")
                        qrow = prow.tile([P, 512], BF16, name="aqrow")
                        nc.vector.tensor_copy(qrow, ps)
                        nc.sync.dma_start(lat_in[:, fc - 6 * half, :], qrow)
                    nc.gpsimd.collective_compute(
                        "AllGather", mybir.AluOpType.bypass,
                        replica_groups=GROUPS,
                        ins=[lat_in[:]], outs=[lat_all[:]],
                    )

        # ---------- Phase B: rope-k + kv_up (latents already normalized) ----------
        with (
            tc.tile_pool(name="pb", bufs=1) as pb,
            tc.tile_pool(name="pb_tmp", bufs=2) as pbt,
            tc.tile_pool(name="pb_ps", bufs=3, space="PSUM") as pps2,
        ):
            with nc.named_scope("phaseB"):
                wk_sb = pb.tile([P, NKC, 512], BF16)
                wv_sb = pb.tile([P, NKC, 512], BF16)
                nc.scalar.dma_start(wk_sb, wkup_in)
                nc.scalar.dma_start(wv_sb, wvup_in)
                kv_sb = pb.tile([P, 6, S], BF16)
                for tt in range(NTT):
                    nc.sync.dma_start(kv_sb[:, :, ts(tt, 512)], latkv_all[tt])
                # roped shared key -> [k;0] and [0;k]
                krd = pb.tile([P, S], BF16)
                t1 = pbt.tile([P, S], BF16, name="bt1")
                nc.vector.tensor_mul(t1, kv_sb[:, 4, :], cos_sb)
                nc.vector.tensor_mul(krd, kv_sb[:, 5, :], sin_sb)
                nc.vector.tensor_add(krd, krd, t1)
                nc.vector.tensor_copy(kre_sb, krd)
                nc.vector.tensor_scalar_mul(kre_sb[64:128, :], kre_sb[64:128, :], 0.0)
                nc.vector.tensor_copy(kro_sb, krd)
                nc.vector.tensor_scalar_mul(kro_sb[0:64, :], kro_sb[0:64, :], 0.0)
                # kv_up: kt (4 head chunks) and V (16 token chunks)
                for d in range(4):
                    for tt in range(NTT):
                        ps = pps2.tile([P, 512], F32, name="bps")
                        for fc in range(NKC):
                            nc.tensor.matmul(
                                ps, wk_sb[:, fc, ds(d * P, P)],
                                kv_sb[:, fc, ts(tt, 512)],
                                start=(fc == 0), stop=(fc == NKC - 1),
                            )
                        nc.scalar.activation(kt_sb[:, d, ts(tt, 512)], ps, AF.Copy)
                for tch in range(NTC):
                    ps = pps2.tile([P, 512], F32, name="bpsv")
                    for fc in range(NKC):
                        nc.tensor.matmul(
                            ps, kv_sb[:, fc, ds(tch * P, P)], wv_sb[:, fc, :],
                            start=(fc == 0), stop=(fc == NKC - 1),
                        )
                    nc.scalar.activation(v_sb[:, tch, :], ps, AF.Copy)

        # ---------- Phases C+D+F interleaved per 512-token tile ----------
        with (
            tc.tile_pool(name="pc_w", bufs=1) as pcw,
            tc.tile_pool(name="pc_slab", bufs=2) as pcs,
            tc.tile_pool(name="pc_tmp", bufs=2) as pct,
            tc.tile_pool(name="pc_ps", bufs=2, space="PSUM") as pps3,
            tc.tile_pool(name="pd_e", bufs=4) as pde,
            tc.tile_pool(name="pd_t", bufs=2) as pdt,
            tc.tile_pool(name="pd_psc", bufs=3, space="PSUM") as pdsc,
            tc.tile_pool(name="pd_pso", bufs=2, space="PSUM") as pdo,
            tc.tile_pool(name="pf_w", bufs=1) as pfw,
            tc.tile_pool(name="pf_row", bufs=2) as pfr,
            tc.tile_pool(name="pf_ps", bufs=1, space="PSUM") as pfp,
        ):
            wq_sb = pcw.tile([P, NQC, NDQ * P], BF16)
            nc.sync.dma_start(wq_sb, wqup_in)
            wo_sb = pfw.tile([P, 4, HID], BF16)
            nc.scalar.dma_start(wo_sb, wo_in)

            def load_slab(i):
                slab = pcs.tile([P, NQC, 512], BF16, name="qslabin")
                nc.sync.dma_start(slab[:, 0:6, :], latq_all_a[i])
                nc.sync.dma_start(slab[:, 6:12, :], latq_all_b[i])
                return slab

            slabs = {0: load_slab(0), 1: load_slab(1)}
            for i in range(NTT):
                tts = ts(i, 512)
                # --- C(i): q_up + q-norm (fused into evictions) + rope-q
                with nc.named_scope(f"phaseC{i}"):
                    if i + 2 < NTT:
                        slabs[i + 2] = load_slab(i + 2)
                    slab = slabs.pop(i)
                    # rq = SCALE / rms(qlat), from the raw gathered latents
                    acc = pct.tile([P, 512], BF16, name="cacc")
                    nc.scalar.square(acc, slab[:, 0, :])
                    for fc in range(1, NQC):
                        sq = pct.tile([P, 512], BF16, name="csq")
                        nc.scalar.square(sq, slab[:, fc, :])
                        nc.vector.tensor_add(acc, acc, sq)
                    rq_b = pcs.tile([P, 512], F32, name="crqb")
                    nc.gpsimd.partition_all_reduce(
                        rq_b, acc, channels=P, reduce_op=bass_isa.ReduceOp.add)
                    nc.scalar.activation(rq_b, rq_b, AF.Sqrt, bias=eps_q,
                                         scale=1.0 / (QR * SCALE * SCALE))
                    nc.vector.reciprocal(rq_b, rq_b)
                    rp = []
                    for d in range(NDQ):
                        ps = pps3.tile([P, 512], F32, name="cps")
                        for fc in range(NQC):
                            nc.tensor.matmul(
                                ps, wq_sb[:, fc, ds(d * P, P)], slab[:, fc, :],
                                start=(fc == 0), stop=(fc == NQC - 1),
                            )
                        if d < 4:
                            nc.vector.tensor_mul(qT[:, d, tts], ps, rq_b)
                        else:
                            r = pct.tile([P, 512], BF16, name=f"rp{d - 4}")
                            nc.vector.tensor_copy(r, ps)
                            rp.append(r)
                    for pr in range(2):
                        ps = pps3.tile([P, 512], F32, name="cps")
                        nc.tensor.matmul(ps, rotm, rp[pr], start=True, stop=True)
                        rr = pct.tile([P, 512], BF16, name="crr")
                        t1 = pct.tile([P, 512], BF16, name="ct1")
                        nc.vector.tensor_copy(rr, ps)
                        nc.vector.tensor_mul(t1, rp[pr], cos_sb[:, tts])
                        nc.vector.tensor_mul(rr, rr, sin_sb[:, tts])
                        nc.vector.tensor_add(t1, t1, rr)
                        nc.vector.tensor_mul(qT[:, 4 + pr, tts], t1, rq_b)

                # --- D(i): attention rows for queries in this tile
                with nc.named_scope(f"phaseD{i}"):
                    jmax = 4 * i + 3
                    # heads processed in even/odd pairs: the pair's score
                    # matmuls interleave on the PE so each AV matmul trails
                    # its exp() by ~850ns of independent work (covers the
                    # mask->exp feeder latency instead of stalling the PE)
                    for hp in range(2):
                        pair = (2 * hp, 2 * hp + 1)
                        qp = qT[:, 4 + hp, :]
                        ps_o = {}
                        eacc = {}
                        for h in pair:
                            ps_o[h] = pdo.tile([P, 512], F32, name="pso")
                            eacc[h] = pdt.tile([P, 512], BF16, name="eacc")
                        for jc in range(jmax + 1):
                            # diagonal tiles only touch the causally-valid
                            # query range [128m, 512); the rest is never read
                            m = jc - 4 * i
                            lo = 128 * m if m > 0 else 0
                            L = 512 - lo
                            cq = ds(lo, L)
                            qcq = ds(512 * i + lo, L)
                            et = {}
                            for h in pair:
                                krop = kre_sb if h % 2 == 0 else kro_sb
                                ps_sc = pdsc.tile([P, 512], F32, name="psc")
                                nc.tensor.matmul(
                                    ps_sc[:, cq], kt_sb[:, h, ds(jc * P, P)],
                                    qT[:, h, qcq], start=True, stop=False)
                                nc.tensor.matmul(
                                    ps_sc[:, cq], krop[:, ds(jc * P, P)],
                                    qp[:, qcq], start=False, stop=True)
                                if m >= 0:
                                    nc.vector.tensor_add(
                                        ps_sc[:, ds(lo, P)],
                                        ps_sc[:, ds(lo, P)],
                                        masks[m][:, ds(lo, P)])
                                et[h] = pde.tile([P, 512], BF16, name="et")
                                nc.scalar.activation(et[h][:, cq],
                                                     ps_sc[:, cq], AF.Exp)
                            for h in pair:
                                nc.tensor.matmul(
                                    ps_o[h][:, cq], v_sb[:, jc, ds(h * P, P)],
                                    et[h][:, cq],
                                    start=(jc == 0), stop=(jc == jmax))
                                if jc == 0:
                                    nc.gpsimd.tensor_copy(eacc[h], et[h])
                                else:
                                    nc.vector.tensor_add(
                                        eacc[h][:, cq], eacc[h][:, cq],
                                        et[h][:, cq])
                        for h in pair:
                            ar = pdt.tile([P, 512], F32, name="dar")
                            nc.gpsimd.partition_all_reduce(
                                ar, eacc[h], channels=P,
                                reduce_op=bass_isa.ReduceOp.add)
                            nc.vector.reciprocal(ar, ar)
                            nc.vector.tensor_mul(o_sb[:, h, tts], ps_o[h], ar)

                # --- F(i): o_proj partial for this token tile
                with nc.named_scope(f"phaseF{i}"):
                    for tch in range(4 * i, 4 * i + 4):
                        orow = pfr.tile([P, HID], BF16, name="orow")
                        for ct in range(4):
                            ps = pfp.tile([P, 512], F32, name="fps")
                            for hh in range(4):
                                nc.tensor.matmul(
                                    ps, o_sb[:, hh, ds(tch * P, P)],
                                    wo_sb[:, hh, ts(ct, 512)],
                                    start=(hh == 0), stop=(hh == 3),
                                )
                            nc.any.tensor_copy(orow[:, ts(ct, 512)], ps)
                        nc.scalar.dma_start(out_d[ds(tch * P, P), :], orow)


_NC_CACHE = None


def _build_nc():
    global _NC_CACHE
    if _NC_CACHE is None:
        nc = bacc.Bacc("TRN2", target_bir_lowering=False, debug=False,
                       num_devices=8)
        with tile.TileContext(nc) as tc:
            _emit(tc)
        nc.compile()
        _NC_CACHE = nc
    return _NC_CACHE


def _shard_inputs(hidden_states, cos, sin, Wq_down, q_gamma, Wq_up,
                  Wkv_down, kv_gamma, Wkv_up, Wo):
    f32 = np.float32
    hid = np.ascontiguousarray(np.asarray(hidden_states, dtype=f32))
    cos = np.asarray(cos, dtype=f32)
    sin = np.asarray(sin, dtype=f32)
    Wqd = np.asarray(Wq_down, dtype=f32)
    Wkd = np.asarray(Wkv_down, dtype=f32)
    qg = np.asarray(q_gamma, dtype=f32)
    kvg = np.asarray(kv_gamma, dtype=f32)
    Wqu = np.asarray(Wq_up, dtype=f32) * qg[None, :]
    Wku = np.asarray(Wkv_up, dtype=f32) * kvg[None, :]
    Wo = np.asarray(Wo, dtype=f32)

    def bf(x):
        return np.ascontiguousarray(x).astype(BF16_NP)

    # shared: combined down-proj weight (rot rows come from the rotm matmul)
    WqdT = Wqd.T                                   # [HID, QR]
    WckvT = Wkd[:KVR].T                            # [HID, KVR]
    krope = Wkd[KVR:].T                            # [HID, 64]
    WdT = np.concatenate([WqdT, WckvT, krope, krope], 1)  # [HID, 2176]
    wd = bf(WdT.reshape(NHC, P, NFC, P).transpose(2, 1, 0, 3))  # [17,128,16,128]
    # rotate_half operator, stationary layout (rotm[p, j] = R2^T)
    r64 = np.zeros((64, 64), dtype=np.float32)
    r64[0:32, 32:64] = np.eye(32)
    r64[32:64, 0:32] = -np.eye(32)
    rotm = np.zeros((P, P), dtype=np.float32)
    rotm[0:64, 0:64] = r64
    rotm[64:128, 64:128] = r64
    rotm = bf(rotm)

    per_batch = []
    for b in range(B):
        h_sw = hid[b].T.reshape(NHC, P, S).transpose(1, 0, 2)  # [128, 16, 2048]
        cT = cos[b].T                               # [64, S]
        sT = sin[b].T
        cos2 = bf(np.concatenate([cT, cT], 0))
        sin2 = bf(np.concatenate([sT, sT], 0))
        per_batch.append((h_sw, cos2, sin2))

    per_group = []
    for g in range(4):
        bn, br = [], []
        for hl in range(4):
            h = 4 * g + hl
            blk = Wqu[h * QKD:(h + 1) * QKD]       # [192, QR]
            bn.append(blk[:NOPE])
            br.append(blk[NOPE:])
        cols = bn + [np.concatenate([br[0], br[1]], 0),
                     np.concatenate([br[2], br[3]], 0)]
        WquT = np.concatenate(cols, 0).T           # [QR, 768]
        wqup = bf(WquT.reshape(NQC, P, NDQ * P).transpose(1, 0, 2))
        kb, vb = [], []
        for hl in range(4):
            h = 4 * g + hl
            blk = Wku[h * (NOPE + VD):(h + 1) * (NOPE + VD)]
            kb.append(blk[:NOPE])
            vb.append(blk[NOPE:])
        WkuT = np.concatenate(kb, 0).T             # [KVR, 512]
        WvuT = np.concatenate(vb, 0).T
        wkup = bf(WkuT.reshape(NKC, P, 512).transpose(1, 0, 2))
        wvup = bf(WvuT.reshape(NKC, P, 512).transpose(1, 0, 2))
        WoT = Wo[:, g * 512:(g + 1) * 512].T       # [512, HID]
        wo = bf(WoT.reshape(4, P, HID).transpose(1, 0, 2))
        per_group.append((wqup, wkup, wvup, wo))

    in_maps = []
    for c in range(8):
        b, g = c // 4, c % 4
        h_sw, cos2, sin2 = per_batch[b]
        wqup, wkup, wvup, wo = per_group[g]
        in_maps.append({
            "hid": bf(h_sw[:, :, 512 * g:512 * (g + 1)]),
            "cos2": cos2, "sin2": sin2, "wd": wd, "rotm": rotm,
            "wqup": wqup, "wkup": wkup, "wvup": wvup, "wo": wo,
        })
    return in_maps


def kernel(hidden_states, cos, sin, Wq_down, q_gamma, Wq_up,
           Wkv_down, kv_gamma, Wkv_up, Wo, _trace=False):
    nc = _build_nc()
    in_maps = _shard_inputs(hidden_states, cos, sin, Wq_down, q_gamma, Wq_up,
                            Wkv_down, kv_gamma, Wkv_up, Wo)
    res = run_bass_kernel_spmd(nc, in_maps, core_ids=list(range(8)),
                               trace=_trace)
    out = np.zeros((B, S, HID), dtype=np.float32)
    for c in range(8):
        out[c // 4] += np.asarray(res.results[c]["out"], dtype=np.float32)
    if _trace:
        kernel.last_results = res
    return out


# revision 39
# speedup vs baseline: 1.0272x; 1.0131x over previous
"""MLA (Multi-head Latent Attention) Bass/Tile kernel for 8 Trainium2 NeuronCores.

Problem: nn_MultiHeadLatentAttention_81707457839331
  B=2, S=2048, HID=2048, NH=16 heads, NOPE=128, ROPE=64, VD=128, QKD=192,
  KVR=512, QR=1536, fp32.

Sharding (single NEFF, SPMD on 8 cores):
  core c -> batch b = c//4, head group g = c%4 (4 heads each).
  The down-projections are SEQUENCE-PARALLEL within each 4-core batch group:
  core (b, g) computes the fused down-proj (q-latent + c_kv + roped-key rows)
  for its 512-token slice only, applies the RMSNorm scales locally (it owns
  all features of its tokens), writes the normalized latents to DRAM, and a
  4-rank AllGather rebuilds the full-S latents on every core of the group.
  q_up / kv_up / attention / o_proj are head-sharded as before. Each core
  emits a partial o_proj output [S, HID]; the host sums 4 partials per batch.

On-device layout: everything stays in [feature, token] layouts so no
on-device transposes are needed. All matmul operands are bf16 (full PE rate,
half the DMA/SBUF footprint, 2x DVE rate); PSUM accumulation stays fp32.
RMSNorm scales are folded into the latents before the AllGather (and SCALE
into the q-norm), RoPE rotate_half is folded into host-rotated weight
columns, the shared roped key is materialized as [k;0]/[0;k] so rope score
matmuls run at full K=128, softmax is unnormalized with bf16 exp tiles, and
the probability sums ride DVE adds + a GpSimd partition_all_reduce.
Phases C (q_up), D (attention) and F (o_proj) interleave per 512-token tile
to keep the PE continuously busy; K/V/Q/O all stay SBUF-resident.
"""

import numpy as np
import ml_dtypes

import concourse.bass as bass
import concourse.bass_isa as bass_isa
import concourse.mybir as mybir
import concourse.tile as tile
from concourse import bacc
from concourse.bass import ds, ts
from concourse.bass_utils import run_bass_kernel_spmd

F32 = mybir.dt.float32
BF16 = mybir.dt.bfloat16
AF = mybir.ActivationFunctionType
BF16_NP = ml_dtypes.bfloat16

B, S, HID, NH = 2, 2048, 2048, 16
NOPE, ROPE, VD = 128, 64, 128
QKD = NOPE + ROPE
KVR, QR = 512, 1536
EPS = 1e-6
SCALE = QKD ** (-0.5)
P = 128

NHC = HID // P            # 16 hidden chunks
NQC = QR // P             # 12 q-latent chunks
NFC = 17                  # down-proj output chunks (12 qlat + 4 ckv + rope-dup)
NKC = KVR // P            # 4 ckv chunks
NTT = S // 512            # 4 token tiles of 512
NTC = S // P              # 16 token chunks of 128
NDQ = 6                   # q_up output chunks: 4 nope + 2 rope-pairs
NEG = -1e30
GROUPS = [[0, 1, 2, 3], [4, 5, 6, 7]]


def _emit(tc):
    nc = tc.nc
    hid_in = nc.dram_tensor("hid", [P, NHC, 512], BF16, kind="ExternalInput").ap()
    cos_in = nc.dram_tensor("cos2", [P, S], BF16, kind="ExternalInput").ap()
    sin_in = nc.dram_tensor("sin2", [P, S], BF16, kind="ExternalInput").ap()
    wd_in = nc.dram_tensor("wd", [NFC, P, NHC, P], BF16, kind="ExternalInput").ap()
    wqup_in = nc.dram_tensor("wqup", [P, NQC, NDQ * P], BF16,
                             kind="ExternalInput").ap()
    wkup_in = nc.dram_tensor("wkup", [P, NKC, 512], BF16, kind="ExternalInput").ap()
    wvup_in = nc.dram_tensor("wvup", [P, NKC, 512], BF16, kind="ExternalInput").ap()
    wo_in = nc.dram_tensor("wo", [P, 4, HID], BF16, kind="ExternalInput").ap()
    rotm_in = nc.dram_tensor("rotm", [P, P], BF16, kind="ExternalInput").ap()
    out_d = nc.dram_tensor("out", [S, HID], BF16, kind="ExternalOutput").ap()

    with (
        tc.tile_pool(name="const", bufs=1) as constp,
        tc.tile_pool(name="persist", bufs=1) as pers,
        tc.tile_pool(name="dram", bufs=1, space="DRAM") as dramp,
    ):
        eps_kv = constp.tile([P, 1], F32)
        nc.vector.memset(eps_kv, EPS)
        eps_q = constp.tile([P, 1], F32)
        nc.vector.memset(eps_q, EPS / (SCALE * SCALE))
        # 4 causal additive masks: mask_k[p, x] = 0 if x - p - 128k >= 0 else -1e30
        masks = []
        for k in range(4):
            m = constp.tile([P, 512], F32, name=f"mask{k}")
            nc.gpsimd.memset(m, 0.0)
            nc.gpsimd.affine_select(
                out=m, in_=m, pattern=[[1, 512]],
                compare_op=mybir.AluOpType.is_ge, fill=NEG,
                base=-128 * k, channel_multiplier=-1,
            )
            masks.append(m)
        cos_sb = constp.tile([P, S], BF16)
        sin_sb = constp.tile([P, S], BF16)
        nc.scalar.dma_start(cos_sb, cos_in)
        nc.scalar.dma_start(sin_sb, sin_in)
        # rotate_half as a K=128 matmul: rot(x) = rotm^T @ x (per 64-block)
        rotm = constp.tile([P, P], BF16)
        nc.scalar.dma_start(rotm, rotm_in)

        # SBUF-resident key/value/query/output tensors (bf16)
        kt_sb = pers.tile([P, 4, S], BF16)     # 4 heads' k_nope.T
        kre_sb = pers.tile([P, S], BF16)       # [k_roped; 0]
        kro_sb = pers.tile([P, S], BF16)       # [0; k_roped]
        v_sb = pers.tile([P, NTC, 512], BF16)  # V in [token, 4*VD]
        qT = pers.tile([P, 6, S], BF16)        # 4 nope + 2 roped pairs
        o_sb = pers.tile([P, 4, S], BF16)      # attention out, head-major

        # DRAM staging for the sequence-parallel down-proj + AllGather.
        # q latents are gathered RAW in two halves (pipelined collectives);
        # their RMSNorm happens in phase C, fused into the psum evictions.
        latq_in_a = dramp.tile([P, 6, 512], BF16)
        latq_in_b = dramp.tile([P, 6, 512], BF16)
        latkv_in = dramp.tile([P, 6, 512], BF16)
        latq_all_a = dramp.tile([NTT, P, 6, 512], BF16)
        latq_all_b = dramp.tile([NTT, P, 6, 512], BF16)
        latkv_all = dramp.tile([NTT, P, 6, 512], BF16)

        # ---------- Phase A: seq-parallel fused down-proj + norms + AG ----------
        with (
            tc.tile_pool(name="pa_hid", bufs=1) as ph,
            tc.tile_pool(name="pa_lat", bufs=1) as plat,
            tc.tile_pool(name="pa_w", bufs=3) as pw,
            tc.tile_pool(name="pa_row", bufs=3) as prow,
            tc.tile_pool(name="pa_tmp", bufs=2) as pat,
            tc.tile_pool(name="pa_ps", bufs=3, space="PSUM") as pps,
        ):
            with nc.named_scope("phaseA"):
                hid_sb = ph.tile([P, NHC, 512], BF16)
                # first weight slice ahead of the 2MB hid load so the first
                # psum group can start as soon as hid chunk 0 lands
                w_first = pw.tile([P, NHC, P], BF16, name="wslice")
                nc.sync.dma_start(w_first, wd_in[12])
                for q in range(4):
                    nc.sync.dma_start(hid_sb[:, 4 * q:4 * q + 4, :],
                                      hid_in[:, 4 * q:4 * q + 4, :])
                ckv_sb = plat.tile([P, NKC, 512], BF16)

                def dp_row(fc, ps_name):
                    if fc == 12:
                        w_sb = w_first
                    else:
                        w_sb = pw.tile([P, NHC, P], BF16, name="wslice")
                        nc.sync.dma_start(w_sb, wd_in[fc])
                    ps = pps.tile([P, 512], F32, name=ps_name)
                    for hc in range(NHC):
                        nc.tensor.matmul(
                            ps, w_sb[:, hc, :], hid_sb[:, hc, :],
                            start=(hc == 0), stop=(hc == NHC - 1),
                        )
                    return ps

                # ckv rows, then their norm while rope/rot rows run on PE
                for fc in range(12, 16):
                    ps = dp_row(fc, "aps")
                    nc.vector.tensor_copy(ckv_sb[:, fc - 12, :], ps)
                acc = pat.tile([P, 512], F32, name="aacc")
                nc.scalar.square(acc, ckv_sb[:, 0, :])
                for fc in range(1, NKC):
                    sq = pat.tile([P, 512], F32, name="asq")
                    nc.scalar.square(sq, ckv_sb[:, fc, :])
                    nc.vector.tensor_add(acc, acc, sq)
                ar = pat.tile([P, 512], F32, name="aar")
                nc.gpsimd.partition_all_reduce(ar, acc, channels=P,
                                               reduce_op=bass_isa.ReduceOp.add)
                nc.scalar.activation(ar, ar, AF.Sqrt, bias=eps_kv, scale=1.0 / KVR)
                nc.vector.reciprocal(ar, ar)
                ps = dp_row(16, "aps")
                row = prow.tile([P, 512], BF16, name="arow")
                nc.vector.tensor_copy(row, ps)
                nc.sync.dma_start(latkv_in[:, 4, :], row)
                ps_rot = pps.tile([P, 512], F32, name="apsrot")
                nc.tensor.matmul(ps_rot, rotm, row, start=True, stop=True)
                rrow = prow.tile([P, 512], BF16, name="arrow")
                nc.vector.tensor_copy(rrow, ps_rot)
                nc.sync.dma_start(latkv_in[:, 5, :], rrow)
                for fc in range(NKC):
                    crow = prow.tile([P, 512], BF16, name="acrow")
                    nc.vector.tensor_mul(crow, ckv_sb[:, fc, :], ar)
                    nc.sync.dma_start(latkv_in[:, fc, :], crow)
                nc.gpsimd.collective_compute(
                    "AllGather", mybir.AluOpType.bypass, replica_groups=GROUPS,
                    ins=[latkv_in[:]], outs=[latkv_all[:]],
                )
                # raw q rows; two pipelined AllGather halves
                for half, lat_in, lat_all in ((0, latq_in_a, latq_all_a),
                                              (1, latq_in_b, latq_all_b)):
                    for fc in range(6 * half, 6 * half + 6):
                        ps = dp_row(fc, "aps")
                        qrow = prow.tile([P, 512], BF16, name="aqrow")
                        nc.vector.tensor_copy(qrow, ps)
                        nc.sync.dma_start(lat_in[:, fc - 6 * half, :], qrow)
                    nc.gpsimd.collective_compute(
                        "AllGather", mybir.AluOpType.bypass,
                        replica_groups=GROUPS,
                        ins=[lat_in[:]], outs=[lat_all[:]],
                    )

        # ---------- Phase B: rope-k + kv_up (latents already normalized) ----------
        with (
            tc.tile_pool(name="pb", bufs=1) as pb,
            tc.tile_pool(name="pb_tmp", bufs=2) as pbt,
            tc.tile_pool(name="pb_ps", bufs=3, space="PSUM") as pps2,
        ):
            with nc.named_scope("phaseB"):
                wk_sb = pb.tile([P, NKC, 512], BF16)
                wv_sb = pb.tile([P, NKC, 512], BF16)
                nc.scalar.dma_start(wk_sb, wkup_in)
                nc.scalar.dma_start(wv_sb, wvup_in)
                kv_sb = pb.tile([P, 6, S], BF16)
                for tt in range(NTT):
                    nc.sync.dma_start(kv_sb[:, :, ts(tt, 512)], latkv_all[tt])
                # roped shared key -> [k;0] and [0;k]
                krd = pb.tile([P, S], BF16)
                t1 = pbt.tile([P, S], BF16, name="bt1")
                nc.vector.tensor_mul(t1, kv_sb[:, 4, :], cos_sb)
                nc.vector.tensor_mul(krd, kv_sb[:, 5, :], sin_sb)
                nc.vector.tensor_add(krd, krd, t1)
                nc.vector.tensor_copy(kre_sb, krd)
                nc.vector.tensor_scalar_mul(kre_sb[64:128, :], kre_sb[64:128, :], 0.0)
                nc.vector.tensor_copy(kro_sb, krd)
                nc.vector.tensor_scalar_mul(kro_sb[0:64, :], kro_sb[0:64, :], 0.0)
                # kv_up: kt + V interleaved per token tile so PE work is
                # available as soon as the first kv_sb tile load lands
                for tt in range(NTT):
                    for d in range(4):
                        ps = pps2.tile([P, 512], F32, name="bps")
                        for fc in range(NKC):
                            nc.tensor.matmul(
                                ps, wk_sb[:, fc, ds(d * P, P)],
                                kv_sb[:, fc, ts(tt, 512)],
                                start=(fc == 0), stop=(fc == NKC - 1),
                            )
                        nc.scalar.activation(kt_sb[:, d, ts(tt, 512)], ps, AF.Copy)
                    for tch in range(4 * tt, 4 * tt + 4):
                        ps = pps2.tile([P, 512], F32, name="bpsv")
                        for fc in range(NKC):
                            nc.tensor.matmul(
                                ps, kv_sb[:, fc, ds(tch * P, P)], wv_sb[:, fc, :],
                                start=(fc == 0), stop=(fc == NKC - 1),
                            )
                        nc.scalar.activation(v_sb[:, tch, :], ps, AF.Copy)

        # ---------- Phases C+D+F interleaved per 512-token tile ----------
        with (
            tc.tile_pool(name="pc_w", bufs=1) as pcw,
            tc.tile_pool(name="pc_slab", bufs=2) as pcs,
            tc.tile_pool(name="pc_tmp", bufs=2) as pct,
            tc.tile_pool(name="pc_ps", bufs=2, space="PSUM") as pps3,
            tc.tile_pool(name="pd_e", bufs=4) as pde,
            tc.tile_pool(name="pd_t", bufs=2) as pdt,
            tc.tile_pool(name="pd_psc", bufs=3, space="PSUM") as pdsc,
            tc.tile_pool(name="pd_pso", bufs=2, space="PSUM") as pdo,
            tc.tile_pool(name="pf_w", bufs=1) as pfw,
            tc.tile_pool(name="pf_row", bufs=2) as pfr,
            tc.tile_pool(name="pf_ps", bufs=1, space="PSUM") as pfp,
        ):
            wq_sb = pcw.tile([P, NQC, NDQ * P], BF16)
            nc.sync.dma_start(wq_sb, wqup_in)
            wo_sb = pfw.tile([P, 4, HID], BF16)
            nc.scalar.dma_start(wo_sb, wo_in)

            def load_slab(i):
                slab = pcs.tile([P, NQC, 512], BF16, name="qslabin")
                nc.sync.dma_start(slab[:, 0:6, :], latq_all_a[i])
                nc.sync.dma_start(slab[:, 6:12, :], latq_all_b[i])
                return slab

            slabs = {0: load_slab(0), 1: load_slab(1)}
            for i in range(NTT):
                tts = ts(i, 512)
                # --- C(i): q_up + q-norm (fused into evictions) + rope-q
                with nc.named_scope(f"phaseC{i}"):
                    if i + 2 < NTT:
                        slabs[i + 2] = load_slab(i + 2)
                    slab = slabs.pop(i)
                    # rq = SCALE / rms(qlat), from the raw gathered latents
                    acc = pct.tile([P, 512], BF16, name="cacc")
                    nc.scalar.square(acc, slab[:, 0, :])
                    for fc in range(1, NQC):
                        sq = pct.tile([P, 512], BF16, name="csq")
                        nc.scalar.square(sq, slab[:, fc, :])
                        nc.vector.tensor_add(acc, acc, sq)
                    rq_b = pcs.tile([P, 512], F32, name="crqb")
                    nc.gpsimd.partition_all_reduce(
                        rq_b, acc, channels=P, reduce_op=bass_isa.ReduceOp.add)
                    nc.scalar.activation(rq_b, rq_b, AF.Sqrt, bias=eps_q,
                                         scale=1.0 / (QR * SCALE * SCALE))
                    nc.vector.reciprocal(rq_b, rq_b)
                    rp = []
                    for d in range(NDQ):
                        ps = pps3.tile([P, 512], F32, name="cps")
                        for fc in range(NQC):
                            nc.tensor.matmul(
                                ps, wq_sb[:, fc, ds(d * P, P)], slab[:, fc, :],
                                start=(fc == 0), stop=(fc == NQC - 1),
                            )
                        if d < 4:
                            nc.vector.tensor_mul(qT[:, d, tts], ps, rq_b)
                        else:
                            r = pct.tile([P, 512], BF16, name=f"rp{d - 4}")
                            nc.vector.tensor_copy(r, ps)
                            rp.append(r)
                    for pr in range(2):
                        ps = pps3.tile([P, 512], F32, name="cps")
                        nc.tensor.matmul(ps, rotm, rp[pr], start=True, stop=True)
                        rr = pct.tile([P, 512], BF16, name="crr")
                        t1 = pct.tile([P, 512], BF16, name="ct1")
                        nc.vector.tensor_copy(rr, ps)
                        nc.vector.tensor_mul(t1, rp[pr], cos_sb[:, tts])
                        nc.vector.tensor_mul(rr, rr, sin_sb[:, tts])
                        nc.vector.tensor_add(t1, t1, rr)
                        nc.vector.tensor_mul(qT[:, 4 + pr, tts], t1, rq_b)

                # --- D(i): attention rows for queries in this tile
                with nc.named_scope(f"phaseD{i}"):
                    jmax = 4 * i + 3
                    # heads processed in even/odd pairs: the pair's score
                    # matmuls interleave on the PE so each AV matmul trails
                    # its exp() by ~850ns of independent work (covers the
                    # mask->exp feeder latency instead of stalling the PE)
                    for hp in range(2):
                        pair = (2 * hp, 2 * hp + 1)
                        qp = qT[:, 4 + hp, :]
                        ps_o = {}
                        eacc = {}
                        for h in pair:
                            ps_o[h] = pdo.tile([P, 512], F32, name="pso")
                            eacc[h] = pdt.tile([P, 512], BF16, name="eacc")
                        for jc in range(jmax + 1):
                            # diagonal tiles only touch the causally-valid
                            # query range [128m, 512); the rest is never read
                            m = jc - 4 * i
                            lo = 128 * m if m > 0 else 0
                            L = 512 - lo
                            cq = ds(lo, L)
                            qcq = ds(512 * i + lo, L)
                            et = {}
                            for h in pair:
                                krop = kre_sb if h % 2 == 0 else kro_sb
                                ps_sc = pdsc.tile([P, 512], F32, name="psc")
                                nc.tensor.matmul(
                                    ps_sc[:, cq], kt_sb[:, h, ds(jc * P, P)],
                                    qT[:, h, qcq], start=True, stop=False)
                                nc.tensor.matmul(
                                    ps_sc[:, cq], krop[:, ds(jc * P, P)],
                                    qp[:, qcq], start=False, stop=True)
                                if m >= 0:
                                    nc.vector.tensor_add(
                                        ps_sc[:, ds(lo, P)],
                                        ps_sc[:, ds(lo, P)],
                                        masks[m][:, ds(lo, P)])
                                et[h] = pde.tile([P, 512], BF16, name="et")
                                nc.scalar.activation(et[h][:, cq],
                                                     ps_sc[:, cq], AF.Exp)
                            for h in pair:
                                nc.tensor.matmul(
                                    ps_o[h][:, cq], v_sb[:, jc, ds(h * P, P)],
                                    et[h][:, cq],
                                    start=(jc == 0), stop=(jc == jmax))
                                if jc == 0:
                                    nc.gpsimd.tensor_copy(eacc[h], et[h])
                                else:
                                    nc.vector.tensor_add(
                                        eacc[h][:, cq], eacc[h][:, cq],
                                        et[h][:, cq])
                        for h in pair:
                            ar = pdt.tile([P, 512], F32, name="dar")
                            nc.gpsimd.partition_all_reduce(
                                ar, eacc[h], channels=P,
                                reduce_op=bass_isa.ReduceOp.add)
                            nc.vector.reciprocal(ar, ar)
                            nc.vector.tensor_mul(o_sb[:, h, tts], ps_o[h], ar)

                # --- F(i): o_proj partial for this token tile
                with nc.named_scope(f"phaseF{i}"):
                    for tch in range(4 * i, 4 * i + 4):
                        orow = pfr.tile([P, HID], BF16, name="orow")
                        for ct in range(4):
                            ps = pfp.tile([P, 512], F32, name="fps")
                            for hh in range(4):
                                nc.tensor.matmul(
                                    ps, o_sb[:, hh, ds(tch * P, P)],
                                    wo_sb[:, hh, ts(ct, 512)],
                                    start=(hh == 0), stop=(hh == 3),
                                )
                            nc.any.tensor_copy(orow[:, ts(ct, 512)], ps)
                        nc.scalar.dma_start(out_d[ds(tch * P, P), :], orow)


_NC_CACHE = None


def _build_nc():
    global _NC_CACHE
    if _NC_CACHE is None:
        nc = bacc.Bacc("TRN2", target_bir_lowering=False, debug=False,
                       num_devices=8)
        with tile.TileContext(nc) as tc:
            _emit(tc)
        nc.compile()
        _NC_CACHE = nc
    return _NC_CACHE


def _shard_inputs(hidden_states, cos, sin, Wq_down, q_gamma, Wq_up,
                  Wkv_down, kv_gamma, Wkv_up, Wo):
    f32 = np.float32
    hid = np.ascontiguousarray(np.asarray(hidden_states, dtype=f32))
    cos = np.asarray(cos, dtype=f32)
    sin = np.asarray(sin, dtype=f32)
    Wqd = np.asarray(Wq_down, dtype=f32)
    Wkd = np.asarray(Wkv_down, dtype=f32)
    qg = np.asarray(q_gamma, dtype=f32)
    kvg = np.asarray(kv_gamma, dtype=f32)
    Wqu = np.asarray(Wq_up, dtype=f32) * qg[None, :]
    Wku = np.asarray(Wkv_up, dtype=f32) * kvg[None, :]
    Wo = np.asarray(Wo, dtype=f32)

    def bf(x):
        return np.ascontiguousarray(x).astype(BF16_NP)

    # shared: combined down-proj weight (rot rows come from the rotm matmul)
    WqdT = Wqd.T                                   # [HID, QR]
    WckvT = Wkd[:KVR].T                            # [HID, KVR]
    krope = Wkd[KVR:].T                            # [HID, 64]
    WdT = np.concatenate([WqdT, WckvT, krope, krope], 1)  # [HID, 2176]
    wd = bf(WdT.reshape(NHC, P, NFC, P).transpose(2, 1, 0, 3))  # [17,128,16,128]
    # rotate_half operator, stationary layout (rotm[p, j] = R2^T)
    r64 = np.zeros((64, 64), dtype=np.float32)
    r64[0:32, 32:64] = np.eye(32)
    r64[32:64, 0:32] = -np.eye(32)
    rotm = np.zeros((P, P), dtype=np.float32)
    rotm[0:64, 0:64] = r64
    rotm[64:128, 64:128] = r64
    rotm = bf(rotm)

    per_batch = []
    for b in range(B):
        h_sw = hid[b].T.reshape(NHC, P, S).transpose(1, 0, 2)  # [128, 16, 2048]
        cT = cos[b].T                               # [64, S]
        sT = sin[b].T
        cos2 = bf(np.concatenate([cT, cT], 0))
        sin2 = bf(np.concatenate([sT, sT], 0))
        per_batch.append((h_sw, cos2, sin2))

    per_group = []
    for g in range(4):
        bn, br = [], []
        for hl in range(4):
            h = 4 * g + hl
            blk = Wqu[h * QKD:(h + 1) * QKD]       # [192, QR]
            bn.append(blk[:NOPE])
            br.append(blk[NOPE:])
        cols = bn + [np.concatenate([br[0], br[1]], 0),
                     np.concatenate([br[2], br[3]], 0)]
        WquT = np.concatenate(cols, 0).T           # [QR, 768]
        wqup = bf(WquT.reshape(NQC, P, NDQ * P).transpose(1, 0, 2))
        kb, vb = [], []
        for hl in range(4):
            h = 4 * g + hl
            blk = Wku[h * (NOPE + VD):(h + 1) * (NOPE + VD)]
            kb.append(blk[:NOPE])
            vb.append(blk[NOPE:])
        WkuT = np.concatenate(kb, 0).T             # [KVR, 512]
        WvuT = np.concatenate(vb, 0).T
        wkup = bf(WkuT.reshape(NKC, P, 512).transpose(1, 0, 2))
        wvup = bf(WvuT.reshape(NKC, P, 512).transpose(1, 0, 2))
        WoT = Wo[:, g * 512:(g + 1) * 512].T       # [512, HID]
        wo = bf(WoT.reshape(4, P, HID).transpose(1, 0, 2))
        per_group.append((wqup, wkup, wvup, wo))

    in_maps = []
    for c in range(8):
        b, g = c // 4, c % 4
        h_sw, cos2, sin2 = per_batch[b]
        wqup, wkup, wvup, wo = per_group[g]
        in_maps.append({
            "hid": bf(h_sw[:, :, 512 * g:512 * (g + 1)]),
            "cos2": cos2, "sin2": sin2, "wd": wd, "rotm": rotm,
            "wqup": wqup, "wkup": wkup, "wvup": wvup, "wo": wo,
        })
    return in_maps


def kernel(hidden_states, cos, sin, Wq_down, q_gamma, Wq_up,
           Wkv_down, kv_gamma, Wkv_up, Wo, _trace=False):
    nc = _build_nc()
    in_maps = _shard_inputs(hidden_states, cos, sin, Wq_down, q_gamma, Wq_up,
                            Wkv_down, kv_gamma, Wkv_up, Wo)
    res = run_bass_kernel_spmd(nc, in_maps, core_ids=list(range(8)),
                               trace=_trace)
    out = np.zeros((B, S, HID), dtype=np.float32)
    for c in range(8):
        out[c // 4] += np.asarray(res.results[c]["out"], dtype=np.float32)
    if _trace:
        kernel.last_results = res
    return out


# revision 41
# speedup vs baseline: 1.0276x; 1.0004x over previous
"""MLA (Multi-head Latent Attention) Bass/Tile kernel for 8 Trainium2 NeuronCores.

Problem: nn_MultiHeadLatentAttention_81707457839331
  B=2, S=2048, HID=2048, NH=16 heads, NOPE=128, ROPE=64, VD=128, QKD=192,
  KVR=512, QR=1536, fp32.

Sharding (single NEFF, SPMD on 8 cores):
  core c -> batch b = c//4, head group g = c%4 (4 heads each).
  The down-projections are SEQUENCE-PARALLEL within each 4-core batch group:
  core (b, g) computes the fused down-proj (q-latent + c_kv + roped-key rows)
  for its 512-token slice only, applies the RMSNorm scales locally (it owns
  all features of its tokens), writes the normalized latents to DRAM, and a
  4-rank AllGather rebuilds the full-S latents on every core of the group.
  q_up / kv_up / attention / o_proj are head-sharded as before. Each core
  emits a partial o_proj output [S, HID]; the host sums 4 partials per batch.

On-device layout: everything stays in [feature, token] layouts so no
on-device transposes are needed. All matmul operands are bf16 (full PE rate,
half the DMA/SBUF footprint, 2x DVE rate); PSUM accumulation stays fp32.
RMSNorm scales are folded into the latents before the AllGather (and SCALE
into the q-norm), RoPE rotate_half is folded into host-rotated weight
columns, the shared roped key is materialized as [k;0]/[0;k] so rope score
matmuls run at full K=128, softmax is unnormalized with bf16 exp tiles, and
the probability sums ride DVE adds + a GpSimd partition_all_reduce.
Phases C (q_up), D (attention) and F (o_proj) interleave per 512-token tile
to keep the PE continuously busy; K/V/Q/O all stay SBUF-resident.
"""

import numpy as np
import ml_dtypes

import concourse.bass as bass
import concourse.bass_isa as bass_isa
import concourse.mybir as mybir
import concourse.tile as tile
from concourse import bacc
from concourse.bass import ds, ts
from concourse.bass_utils import run_bass_kernel_spmd

F32 = mybir.dt.float32
BF16 = mybir.dt.bfloat16
AF = mybir.ActivationFunctionType
BF16_NP = ml_dtypes.bfloat16

B, S, HID, NH = 2, 2048, 2048, 16
NOPE, ROPE, VD = 128, 64, 128
QKD = NOPE + ROPE
KVR, QR = 512, 1536
EPS = 1e-6
SCALE = QKD ** (-0.5)
P = 128

NHC = HID // P            # 16 hidden chunks
NQC = QR // P             # 12 q-latent chunks
NFC = 17                  # down-proj output chunks (12 qlat + 4 ckv + rope-dup)
NKC = KVR // P            # 4 ckv chunks
NTT = S // 512            # 4 token tiles of 512
NTC = S // P              # 16 token chunks of 128
NDQ = 6                   # q_up output chunks: 4 nope + 2 rope-pairs
NEG = -1e30
GROUPS = [[0, 1, 2, 3], [4, 5, 6, 7]]


def _emit(tc):
    nc = tc.nc
    hid_in = nc.dram_tensor("hid", [P, NHC, 512], BF16, kind="ExternalInput").ap()
    cos_in = nc.dram_tensor("cos2", [P, S], BF16, kind="ExternalInput").ap()
    sin_in = nc.dram_tensor("sin2", [P, S], BF16, kind="ExternalInput").ap()
    wd_in = nc.dram_tensor("wd", [NFC, P, NHC, P], BF16, kind="ExternalInput").ap()
    wqup_in = nc.dram_tensor("wqup", [P, NQC, NDQ * P], BF16,
                             kind="ExternalInput").ap()
    wkup_in = nc.dram_tensor("wkup", [P, NKC, 512], BF16, kind="ExternalInput").ap()
    wvup_in = nc.dram_tensor("wvup", [P, NKC, 512], BF16, kind="ExternalInput").ap()
    wo_in = nc.dram_tensor("wo", [P, 4, HID], BF16, kind="ExternalInput").ap()
    rotm_in = nc.dram_tensor("rotm", [P, P], BF16, kind="ExternalInput").ap()
    out_d = nc.dram_tensor("out", [S, HID], BF16, kind="ExternalOutput").ap()

    with (
        tc.tile_pool(name="const", bufs=1) as constp,
        tc.tile_pool(name="persist", bufs=1) as pers,
        tc.tile_pool(name="dram", bufs=1, space="DRAM") as dramp,
    ):
        eps_kv = constp.tile([P, 1], F32)
        nc.vector.memset(eps_kv, EPS)
        eps_q = constp.tile([P, 1], F32)
        nc.vector.memset(eps_q, EPS / (SCALE * SCALE))
        # 4 causal additive masks: mask_k[p, x] = 0 if x - p - 128k >= 0 else -1e30
        masks = []
        for k in range(4):
            m = constp.tile([P, 512], F32, name=f"mask{k}")
            nc.gpsimd.memset(m, 0.0)
            nc.gpsimd.affine_select(
                out=m, in_=m, pattern=[[1, 512]],
                compare_op=mybir.AluOpType.is_ge, fill=NEG,
                base=-128 * k, channel_multiplier=-1,
            )
            masks.append(m)
        cos_sb = constp.tile([P, S], BF16)
        sin_sb = constp.tile([P, S], BF16)
        nc.scalar.dma_start(cos_sb, cos_in)
        nc.scalar.dma_start(sin_sb, sin_in)
        # rotate_half as a K=128 matmul: rot(x) = rotm^T @ x (per 64-block)
        rotm = constp.tile([P, P], BF16)
        nc.scalar.dma_start(rotm, rotm_in)

        # SBUF-resident key/value/query/output tensors (bf16)
        kt_sb = pers.tile([P, 4, S], BF16)     # 4 heads' k_nope.T
        kre_sb = pers.tile([P, S], BF16)       # [k_roped; 0]
        kro_sb = pers.tile([P, S], BF16)       # [0; k_roped]
        v_sb = pers.tile([P, NTC, 512], BF16)  # V in [token, 4*VD]
        qT = pers.tile([P, 6, S], BF16)        # 4 nope + 2 roped pairs
        o_sb = pers.tile([P, 4, S], BF16)      # attention out, head-major

        # DRAM staging for the sequence-parallel down-proj + AllGather.
        # q latents are gathered RAW in two halves (pipelined collectives);
        # their RMSNorm happens in phase C, fused into the psum evictions.
        latq_in_a = dramp.tile([P, 6, 512], BF16)
        latq_in_b = dramp.tile([P, 6, 512], BF16)
        latkv_in = dramp.tile([P, 6, 512], BF16)
        latq_all_a = dramp.tile([NTT, P, 6, 512], BF16)
        latq_all_b = dramp.tile([NTT, P, 6, 512], BF16)
        latkv_all = dramp.tile([NTT, P, 6, 512], BF16)

        # ---------- Phase A: seq-parallel fused down-proj + norms + AG ----------
        with (
            tc.tile_pool(name="pa_hid", bufs=1) as ph,
            tc.tile_pool(name="pa_lat", bufs=1) as plat,
            tc.tile_pool(name="pa_w", bufs=3) as pw,
            tc.tile_pool(name="pa_row", bufs=3) as prow,
            tc.tile_pool(name="pa_tmp", bufs=2) as pat,
            tc.tile_pool(name="pa_ps", bufs=3, space="PSUM") as pps,
        ):
            with nc.named_scope("phaseA"):
                hid_sb = ph.tile([P, NHC, 512], BF16)
                # first weight slice ahead of the 2MB hid load so the first
                # psum group can start as soon as hid chunk 0 lands
                w_first = pw.tile([P, NHC, P], BF16, name="wslice")
                nc.sync.dma_start(w_first, wd_in[12])
                for q in range(4):
                    nc.sync.dma_start(hid_sb[:, 4 * q:4 * q + 4, :],
                                      hid_in[:, 4 * q:4 * q + 4, :])
                ckv_sb = plat.tile([P, NKC, 512], BF16)

                def dp_row(fc, ps_name):
                    if fc == 12:
                        w_sb = w_first
                    else:
                        w_sb = pw.tile([P, NHC, P], BF16, name="wslice")
                        nc.sync.dma_start(w_sb, wd_in[fc])
                    ps = pps.tile([P, 512], F32, name=ps_name)
                    for hc in range(NHC):
                        nc.tensor.matmul(
                            ps, w_sb[:, hc, :], hid_sb[:, hc, :],
                            start=(hc == 0), stop=(hc == NHC - 1),
                        )
                    return ps

                # ckv rows, then their norm while rope/rot rows run on PE
                for fc in range(12, 16):
                    ps = dp_row(fc, "aps")
                    nc.vector.tensor_copy(ckv_sb[:, fc - 12, :], ps)
                acc = pat.tile([P, 512], F32, name="aacc")
                nc.scalar.square(acc, ckv_sb[:, 0, :])
                for fc in range(1, NKC):
                    sq = pat.tile([P, 512], F32, name="asq")
                    nc.scalar.square(sq, ckv_sb[:, fc, :])
                    nc.vector.tensor_add(acc, acc, sq)
                ar = pat.tile([P, 512], F32, name="aar")
                nc.gpsimd.partition_all_reduce(ar, acc, channels=P,
                                               reduce_op=bass_isa.ReduceOp.add)
                nc.scalar.activation(ar, ar, AF.Sqrt, bias=eps_kv, scale=1.0 / KVR)
                nc.vector.reciprocal(ar, ar)
                ps = dp_row(16, "aps")
                row = prow.tile([P, 512], BF16, name="arow")
                nc.vector.tensor_copy(row, ps)
                nc.sync.dma_start(latkv_in[:, 4, :], row)
                ps_rot = pps.tile([P, 512], F32, name="apsrot")
                nc.tensor.matmul(ps_rot, rotm, row, start=True, stop=True)
                rrow = prow.tile([P, 512], BF16, name="arrow")
                nc.vector.tensor_copy(rrow, ps_rot)
                nc.sync.dma_start(latkv_in[:, 5, :], rrow)
                for fc in range(NKC):
                    crow = prow.tile([P, 512], BF16, name="acrow")
                    nc.vector.tensor_mul(crow, ckv_sb[:, fc, :], ar)
                    nc.sync.dma_start(latkv_in[:, fc, :], crow)
                nc.gpsimd.collective_compute(
                    "AllGather", mybir.AluOpType.bypass, replica_groups=GROUPS,
                    ins=[latkv_in[:]], outs=[latkv_all[:]],
                )
                # raw q rows; two pipelined AllGather halves
                for half, lat_in, lat_all in ((0, latq_in_a, latq_all_a),
                                              (1, latq_in_b, latq_all_b)):
                    for fc in range(6 * half, 6 * half + 6):
                        ps = dp_row(fc, "aps")
                        qrow = prow.tile([P, 512], BF16, name="aqrow")
                        nc.vector.tensor_copy(qrow, ps)
                        nc.sync.dma_start(lat_in[:, fc - 6 * half, :], qrow)
                    nc.gpsimd.collective_compute(
                        "AllGather", mybir.AluOpType.bypass,
                        replica_groups=GROUPS,
                        ins=[lat_in[:]], outs=[lat_all[:]],
                    )

        # ---------- Phase B: rope-k + kv_up (latents already normalized) ----------
        with (
            tc.tile_pool(name="pb", bufs=1) as pb,
            tc.tile_pool(name="pb_tmp", bufs=2) as pbt,
            tc.tile_pool(name="pb_ps", bufs=3, space="PSUM") as pps2,
        ):
            with nc.named_scope("phaseB"):
                wk_sb = pb.tile([P, NKC, 512], BF16)
                wv_sb = pb.tile([P, NKC, 512], BF16)
                nc.scalar.dma_start(wk_sb, wkup_in)
                nc.scalar.dma_start(wv_sb, wvup_in)
                kv_sb = pb.tile([P, 6, S], BF16)
                for tt in range(NTT):
                    nc.sync.dma_start(kv_sb[:, :, ts(tt, 512)], latkv_all[tt])
                # roped shared key -> [k;0] and [0;k]
                krd = pb.tile([P, S], BF16)
                t1 = pbt.tile([P, S], BF16, name="bt1")
                nc.vector.tensor_mul(t1, kv_sb[:, 4, :], cos_sb)
                nc.vector.tensor_mul(krd, kv_sb[:, 5, :], sin_sb)
                nc.vector.tensor_add(krd, krd, t1)
                nc.vector.tensor_copy(kre_sb, krd)
                nc.vector.tensor_scalar_mul(kre_sb[64:128, :], kre_sb[64:128, :], 0.0)
                nc.vector.tensor_copy(kro_sb, krd)
                nc.vector.tensor_scalar_mul(kro_sb[0:64, :], kro_sb[0:64, :], 0.0)
                # kv_up: kt + V interleaved per token tile so PE work is
                # available as soon as the first kv_sb tile load lands
                for tt in range(NTT):
                    for d in range(4):
                        ps = pps2.tile([P, 512], F32, name="bps")
                        for fc in range(NKC):
                            nc.tensor.matmul(
                                ps, wk_sb[:, fc, ds(d * P, P)],
                                kv_sb[:, fc, ts(tt, 512)],
                                start=(fc == 0), stop=(fc == NKC - 1),
                            )
                        nc.scalar.activation(kt_sb[:, d, ts(tt, 512)], ps, AF.Copy)
                    for tch in range(4 * tt, 4 * tt + 4):
                        ps = pps2.tile([P, 512], F32, name="bpsv")
                        for fc in range(NKC):
                            nc.tensor.matmul(
                                ps, kv_sb[:, fc, ds(tch * P, P)], wv_sb[:, fc, :],
                                start=(fc == 0), stop=(fc == NKC - 1),
                            )
                        nc.scalar.activation(v_sb[:, tch, :], ps, AF.Copy)

        # ---------- Phases C+D+F interleaved per 512-token tile ----------
        with (
            tc.tile_pool(name="pc_w", bufs=1) as pcw,
            tc.tile_pool(name="pc_slab", bufs=2) as pcs,
            tc.tile_pool(name="pc_tmp", bufs=2) as pct,
            tc.tile_pool(name="pc_ps", bufs=2, space="PSUM") as pps3,
            tc.tile_pool(name="pd_e", bufs=4) as pde,
            tc.tile_pool(name="pd_t", bufs=2) as pdt,
            tc.tile_pool(name="pd_psc", bufs=3, space="PSUM") as pdsc,
            tc.tile_pool(name="pd_pso", bufs=2, space="PSUM") as pdo,
            tc.tile_pool(name="pf_w", bufs=1) as pfw,
            tc.tile_pool(name="pf_row", bufs=2) as pfr,
            tc.tile_pool(name="pf_ps", bufs=1, space="PSUM") as pfp,
        ):
            wq_sb = pcw.tile([P, NQC, NDQ * P], BF16)
            nc.sync.dma_start(wq_sb, wqup_in)
            wo_sb = pfw.tile([P, 4, HID], BF16)
            nc.scalar.dma_start(wo_sb, wo_in)

            def load_slab(i):
                slab = pcs.tile([P, NQC, 512], BF16, name="qslabin")
                nc.sync.dma_start(slab[:, 0:6, :], latq_all_a[i])
                nc.sync.dma_start(slab[:, 6:12, :], latq_all_b[i])
                return slab

            # descending tile order: the largest attention tile (i=3) fills
            # the pipeline first and the smallest (i=0) drains last
            order = [3, 2, 1, 0]
            slabs = {order[0]: load_slab(order[0]),
                     order[1]: load_slab(order[1])}
            for oi, i in enumerate(order):
                tts = ts(i, 512)
                # --- C(i): q_up + q-norm (fused into evictions) + rope-q
                with nc.named_scope(f"phaseC{i}"):
                    if oi + 2 < NTT:
                        slabs[order[oi + 2]] = load_slab(order[oi + 2])
                    slab = slabs.pop(i)
                    # rq = SCALE / rms(qlat), from the raw gathered latents
                    acc = pct.tile([P, 512], BF16, name="cacc")
                    nc.scalar.square(acc, slab[:, 0, :])
                    for fc in range(1, NQC):
                        sq = pct.tile([P, 512], BF16, name="csq")
                        nc.scalar.square(sq, slab[:, fc, :])
                        nc.vector.tensor_add(acc, acc, sq)
                    rq_b = pcs.tile([P, 512], F32, name="crqb")
                    nc.gpsimd.partition_all_reduce(
                        rq_b, acc, channels=P, reduce_op=bass_isa.ReduceOp.add)
                    nc.scalar.activation(rq_b, rq_b, AF.Sqrt, bias=eps_q,
                                         scale=1.0 / (QR * SCALE * SCALE))
                    nc.vector.reciprocal(rq_b, rq_b)
                    rp = []
                    for d in range(NDQ):
                        ps = pps3.tile([P, 512], F32, name="cps")
                        for fc in range(NQC):
                            nc.tensor.matmul(
                                ps, wq_sb[:, fc, ds(d * P, P)], slab[:, fc, :],
                                start=(fc == 0), stop=(fc == NQC - 1),
                            )
                        if d < 4:
                            nc.vector.tensor_mul(qT[:, d, tts], ps, rq_b)
                        else:
                            r = pct.tile([P, 512], BF16, name=f"rp{d - 4}")
                            nc.vector.tensor_copy(r, ps)
                            rp.append(r)
                    for pr in range(2):
                        ps = pps3.tile([P, 512], F32, name="cps")
                        nc.tensor.matmul(ps, rotm, rp[pr], start=True, stop=True)
                        rr = pct.tile([P, 512], BF16, name="crr")
                        t1 = pct.tile([P, 512], BF16, name="ct1")
                        nc.vector.tensor_copy(rr, ps)
                        nc.vector.tensor_mul(t1, rp[pr], cos_sb[:, tts])
                        nc.vector.tensor_mul(rr, rr, sin_sb[:, tts])
                        nc.vector.tensor_add(t1, t1, rr)
                        nc.vector.tensor_mul(qT[:, 4 + pr, tts], t1, rq_b)

                # --- D(i): attention rows for queries in this tile
                with nc.named_scope(f"phaseD{i}"):
                    jmax = 4 * i + 3
                    # heads processed in even/odd pairs: the pair's score
                    # matmuls interleave on the PE so each AV matmul trails
                    # its exp() by ~850ns of independent work (covers the
                    # mask->exp feeder latency instead of stalling the PE)
                    for hp in range(2):
                        pair = (2 * hp, 2 * hp + 1)
                        qp = qT[:, 4 + hp, :]
                        ps_o = {}
                        eacc = {}
                        for h in pair:
                            ps_o[h] = pdo.tile([P, 512], F32, name="pso")
                            eacc[h] = pdt.tile([P, 512], BF16, name="eacc")
                        for jc in range(jmax + 1):
                            # diagonal tiles only touch the causally-valid
                            # query range [128m, 512); the rest is never read
                            m = jc - 4 * i
                            lo = 128 * m if m > 0 else 0
                            L = 512 - lo
                            cq = ds(lo, L)
                            qcq = ds(512 * i + lo, L)
                            et = {}
                            for h in pair:
                                krop = kre_sb if h % 2 == 0 else kro_sb
                                ps_sc = pdsc.tile([P, 512], F32, name="psc")
                                nc.tensor.matmul(
                                    ps_sc[:, cq], kt_sb[:, h, ds(jc * P, P)],
                                    qT[:, h, qcq], start=True, stop=False)
                                nc.tensor.matmul(
                                    ps_sc[:, cq], krop[:, ds(jc * P, P)],
                                    qp[:, qcq], start=False, stop=True)
                                if m >= 0:
                                    nc.vector.tensor_add(
                                        ps_sc[:, ds(lo, P)],
                                        ps_sc[:, ds(lo, P)],
                                        masks[m][:, ds(lo, P)])
                                et[h] = pde.tile([P, 512], BF16, name="et")
                                nc.scalar.activation(et[h][:, cq],
                                                     ps_sc[:, cq], AF.Exp)
                            for h in pair:
                                nc.tensor.matmul(
                                    ps_o[h][:, cq], v_sb[:, jc, ds(h * P, P)],
                                    et[h][:, cq],
                                    start=(jc == 0), stop=(jc == jmax))
                                if jc == 0:
                                    nc.gpsimd.tensor_copy(eacc[h], et[h])
                                else:
                                    nc.vector.tensor_add(
                                        eacc[h][:, cq], eacc[h][:, cq],
                                        et[h][:, cq])
                        for h in pair:
                            ar = pdt.tile([P, 512], F32, name="dar")
                            nc.gpsimd.partition_all_reduce(
                                ar, eacc[h], channels=P,
                                reduce_op=bass_isa.ReduceOp.add)
                            nc.vector.reciprocal(ar, ar)
                            nc.vector.tensor_mul(o_sb[:, h, tts], ps_o[h], ar)

                # --- F(i): o_proj partial for this token tile
                with nc.named_scope(f"phaseF{i}"):
                    for tch in range(4 * i, 4 * i + 4):
                        orow = pfr.tile([P, HID], BF16, name="orow")
                        for ct in range(4):
                            ps = pfp.tile([P, 512], F32, name="fps")
                            for hh in range(4):
                                nc.tensor.matmul(
                                    ps, o_sb[:, hh, ds(tch * P, P)],
                                    wo_sb[:, hh, ts(ct, 512)],
                                    start=(hh == 0), stop=(hh == 3),
                                )
                            nc.any.tensor_copy(orow[:, ts(ct, 512)], ps)
                        nc.scalar.dma_start(out_d[ds(tch * P, P), :], orow)


_NC_CACHE = None


def _build_nc():
    global _NC_CACHE
    if _NC_CACHE is None:
        nc = bacc.Bacc("TRN2", target_bir_lowering=False, debug=False,
                       num_devices=8)
        with tile.TileContext(nc) as tc:
            _emit(tc)
        nc.compile()
        _NC_CACHE = nc
    return _NC_CACHE


def _shard_inputs(hidden_states, cos, sin, Wq_down, q_gamma, Wq_up,
                  Wkv_down, kv_gamma, Wkv_up, Wo):
    f32 = np.float32
    hid = np.ascontiguousarray(np.asarray(hidden_states, dtype=f32))
    cos = np.asarray(cos, dtype=f32)
    sin = np.asarray(sin, dtype=f32)
    Wqd = np.asarray(Wq_down, dtype=f32)
    Wkd = np.asarray(Wkv_down, dtype=f32)
    qg = np.asarray(q_gamma, dtype=f32)
    kvg = np.asarray(kv_gamma, dtype=f32)
    Wqu = np.asarray(Wq_up, dtype=f32) * qg[None, :]
    Wku = np.asarray(Wkv_up, dtype=f32) * kvg[None, :]
    Wo = np.asarray(Wo, dtype=f32)

    def bf(x):
        return np.ascontiguousarray(x).astype(BF16_NP)

    # shared: combined down-proj weight (rot rows come from the rotm matmul)
    WqdT = Wqd.T                                   # [HID, QR]
    WckvT = Wkd[:KVR].T                            # [HID, KVR]
    krope = Wkd[KVR:].T                            # [HID, 64]
    WdT = np.concatenate([WqdT, WckvT, krope, krope], 1)  # [HID, 2176]
    wd = bf(WdT.reshape(NHC, P, NFC, P).transpose(2, 1, 0, 3))  # [17,128,16,128]
    # rotate_half operator, stationary layout (rotm[p, j] = R2^T)
    r64 = np.zeros((64, 64), dtype=np.float32)
    r64[0:32, 32:64] = np.eye(32)
    r64[32:64, 0:32] = -np.eye(32)
    rotm = np.zeros((P, P), dtype=np.float32)
    rotm[0:64, 0:64] = r64
    rotm[64:128, 64:128] = r64
    rotm = bf(rotm)

    per_batch = []
    for b in range(B):
        h_sw = hid[b].T.reshape(NHC, P, S).transpose(1, 0, 2)  # [128, 16, 2048]
        cT = cos[b].T                               # [64, S]
        sT = sin[b].T
        cos2 = bf(np.concatenate([cT, cT], 0))
        sin2 = bf(np.concatenate([sT, sT], 0))
        per_batch.append((h_sw, cos2, sin2))

    per_group = []
    for g in range(4):
        bn, br = [], []
        for hl in range(4):
            h = 4 * g + hl
            blk = Wqu[h * QKD:(h + 1) * QKD]       # [192, QR]
            bn.append(blk[:NOPE])
            br.append(blk[NOPE:])
        cols = bn + [np.concatenate([br[0], br[1]], 0),
                     np.concatenate([br[2], br[3]], 0)]
        WquT = np.concatenate(cols, 0).T           # [QR, 768]
        wqup = bf(WquT.reshape(NQC, P, NDQ * P).transpose(1, 0, 2))
        kb, vb = [], []
        for hl in range(4):
            h = 4 * g + hl
            blk = Wku[h * (NOPE + VD):(h + 1) * (NOPE + VD)]
            kb.append(blk[:NOPE])
            vb.append(blk[NOPE:])
        WkuT = np.concatenate(kb, 0).T             # [KVR, 512]
        WvuT = np.concatenate(vb, 0).T
        wkup = bf(WkuT.reshape(NKC, P, 512).transpose(1, 0, 2))
        wvup = bf(WvuT.reshape(NKC, P, 512).transpose(1, 0, 2))
        WoT = Wo[:, g * 512:(g + 1) * 512].T       # [512, HID]
        wo = bf(WoT.reshape(4, P, HID).transpose(1, 0, 2))
        per_group.append((wqup, wkup, wvup, wo))

    in_maps = []
    for c in range(8):
        b, g = c // 4, c % 4
        h_sw, cos2, sin2 = per_batch[b]
        wqup, wkup, wvup, wo = per_group[g]
        in_maps.append({
            "hid": bf(h_sw[:, :, 512 * g:512 * (g + 1)]),
            "cos2": cos2, "sin2": sin2, "wd": wd, "rotm": rotm,
            "wqup": wqup, "wkup": wkup, "wvup": wvup, "wo": wo,
        })
    return in_maps


def kernel(hidden_states, cos, sin, Wq_down, q_gamma, Wq_up,
           Wkv_down, kv_gamma, Wkv_up, Wo, _trace=False):
    nc = _build_nc()
    in_maps = _shard_inputs(hidden_states, cos, sin, Wq_down, q_gamma, Wq_up,
                            Wkv_down, kv_gamma, Wkv_up, Wo)
    res = run_bass_kernel_spmd(nc, in_maps, core_ids=list(range(8)),
                               trace=_trace)
    out = np.zeros((B, S, HID), dtype=np.float32)
    for c in range(8):
        out[c // 4] += np.asarray(res.results[c]["out"], dtype=np.float32)
    if _trace:
        kernel.last_results = res
    return out
